# revision 12
# baseline (speedup 1.0000x reference)
"""Trainium2 Bass kernel for nn_Cell_67894843015282 (DARTS-style NAS cell).

Strategy:
  - All routing/gating logic (masks, node_sum chain, sigmoid gates) depends only
    on small parameter tensors -> computed on host in f32 (jax-on-CPU to match
    the reference bit-for-bit on the step() comparisons).
  - BN affine + channel gates + op coefficients folded into the pointwise conv
    matrices (per-output-channel scale) and hoisted bias vectors; ops with a
    zero coefficient are skipped entirely.
  - Depthwise (x) pointwise conv pairs fused into per-tap 128x128 matmuls on the
    tensor engine (fp32r), accumulating across taps/ops/edges directly in PSUM.
    A balance solver spills some taps to the vector engine (per-partition-scalar
    multiply-accumulate chains) with the pointwise applied by one extra matmul.
  - Pools (max/avg 3x3) computed once per source state on the vector engine
    (separable passes); skip/pool contributions accumulated with
    scalar_tensor_tensor into an SBUF accumulator.
  - Data parallel over batch: 1 image per NeuronCore, 8 cores.
"""

import os

import numpy as np

B, C, HH, WW = 8, 128, 32, 32
PIX = HH * WW
C_PREV = 512
STEPS, N_EDGES, N_OPS = 4, 14, 8
N_CORES = 8

# cost-model constants (ns) for the balance solver
PE_TAP = 470.0  # two N=512 fp32r matmuls
PE_MM = 235.0
DVE_STT = 1222.0  # scalar_tensor_tensor on (128,1024)
DVE_TS = 689.0  # tensor_scalar f32 (single-src, 2x) -- unused, tap0 is on ACT
DVE_TT_BIG = 1222.0
DVE_TT_SMALL = 250.0
ACT_OP = 1257.0

# ---------------------------------------------------------------------------
# Host-side gating / fusion (the "plan")
# ---------------------------------------------------------------------------


def _f32(x):
    return np.asarray(x, dtype=np.float32)


def _gate_math(inputs):
    """Replicate the data-independent gating chain of the reference in f32.

    Uses jax on CPU when available so the step() threshold comparisons match
    the reference numerics exactly; falls back to numpy.
    """
    try:
        import jax

        cpu = jax.devices("cpu")[0]

        with jax.default_device(cpu):
            import jax.numpy as jnp

            return _gate_math_impl(jnp, jax.nn.sigmoid, inputs, to_np=np.asarray)
    except Exception:

        def np_sig(x):
            return 1.0 / (1.0 + np.exp(-np.asarray(x, np.float32), dtype=np.float32))

        return _gate_math_impl(np, np_sig, inputs, to_np=np.asarray)


def _gate_math_impl(xp, sig, inputs, to_np):
    f32 = np.float32
    weights2 = xp.asarray(inputs["weights2"], dtype=f32)
    thre = xp.asarray(inputs["thre"], dtype=f32)
    mask_default = xp.asarray(inputs["mask_default"])
    kernel_param = xp.asarray(inputs["kernel_param"], dtype=f32)
    mask_k_default = xp.asarray(inputs["mask_k_default"])
    mask_w_default = xp.asarray(inputs["mask_w_default"])
    kernel_pre = xp.asarray(inputs["kernel_pre"], dtype=f32)
    thre_pre = xp.asarray(inputs["thre_pre"], dtype=f32)

    def step(x):
        return (x > 0).astype(f32)

    mdf = mask_default.astype(f32)

    g0 = sig(kernel_pre[0])
    mk0 = step(g0 - thre_pre[0])
    gv0 = to_np(g0 * mk0).astype(f32)
    g1p = sig(kernel_pre[1])
    mk1 = step(g1p - thre_pre[1])
    gv1 = to_np(g1p * mk1).astype(f32)

    n_states = 2
    offset = 0
    m_all = np.zeros((N_EDGES, N_OPS), np.float32)
    for i in range(STEPS):
        n = n_states
        weight_sum = (weights2[offset : offset + n] * mdf[offset : offset + n]).sum()
        for j in range(n):
            e = offset + j
            ns = weight_sum
            m_list = []
            for k in range(N_OPS):
                w = weights2[e, k]
                md = mdf[e, k]
                m = xp.where(
                    md == 0, f32(0.0), xp.where(w != ns, step(w - thre[e, k, 0]), md)
                )
                cond = (md != 0) & (w != ns) & (m == 0)
                m_list.append(m)
                ns = xp.where(cond, ns - w, ns)
            m_vec = xp.stack(m_list)
            weight_sum = (
                weight_sum - (weights2[e] * mdf[e]).sum() + (weights2[e] * m_vec).sum()
            )
            m_all[e] = to_np(m_vec)
        offset += n
        n_states += 1

    coef = to_np(weights2).astype(f32) * m_all

    gates = to_np(sig(kernel_param)).astype(f32)
    t1 = to_np(thre[:, :, 1]).astype(f32)
    t2 = to_np(thre[:, :, 2]).astype(f32)
    mk = (gates - t1[:, :, None] > 0).astype(f32) * (to_np(mask_k_default) != 0)
    mw = (gates - t2[:, :, None] > 0).astype(f32) * (to_np(mask_w_default) != 0)
    return dict(
        gv0=gv0,
        gv1=gv1,
        coef=coef,
        gates=gates,
        mk=mk.astype(np.float32),
        mw=mw.astype(np.float32),
    )


TAPS3 = [(dy, dx) for dy in (-1, 0, 1) for dx in (-1, 0, 1)]
TAPS5 = [(dy, dx) for dy in (-2, -1, 0, 1, 2) for dx in (-2, -1, 0, 1, 2)]
TAPS3D = [(dy, dx) for dy in (-2, 0, 2) for dx in (-2, 0, 2)]
TAPS5D = [(dy, dx) for dy in (-4, -2, 0, 2, 4) for dx in (-4, -2, 0, 2, 4)]


def build_plan(inputs):
    g = _gate_math(inputs)
    coef = g["coef"]

    scale0 = _f32(inputs["pre0_g"]) * g["gv0"]
    bias0 = _f32(inputs["pre0_b"]) * g["gv0"]
    scale1 = _f32(inputs["pre1_g"]) * g["gv1"]
    bias1 = _f32(inputs["pre1_b"]) * g["gv1"]
    wpre0 = (_f32(inputs["pre0_w"]) * scale0[:, None]).T.copy()  # (512,128)
    wpre1 = (_f32(inputs["pre1_w"]) * scale1[:, None]).T.copy()

    state_of_edge = []
    for i in range(STEPS):
        for j in range(2 + i):
            state_of_edge.append((i, j))

    edges = []
    state_bias = np.zeros((6, C), np.float32)
    for e in range(N_EDGES):
        i, j = state_of_edge[e]
        tgt = 2 + i
        ops = {
            "max": float(coef[e, 1]),
            "avg": float(coef[e, 2]),
            "skip": float(coef[e, 3]),
        }
        for k, nm, taps in ((4, "sep3", TAPS3), (5, "sep5", TAPS5)):
            c = float(coef[e, k])
            if c == 0.0:
                ops[nm] = None
                continue
            gate = g["gates"][e, k]
            mk = g["mk"][e, k]
            mw = g["mw"][e, k]
            s1 = _f32(inputs[f"{nm}_g1"][e]) * gate * mk
            bb1 = _f32(inputs[f"{nm}_b1"][e]) * gate * mk
            s2 = c * _f32(inputs[f"{nm}_g2"][e]) * gate * mw
            bb2 = c * _f32(inputs[f"{nm}_b2"][e]) * gate * mw
            state_bias[tgt] += bb2
            if not s2.any() or not (s1.any() or bb1.any()):
                ops[nm] = None
                continue
            ops[nm] = dict(
                taps=taps,
                layers=[
                    dict(
                        dw=_f32(inputs[f"{nm}_dw1"][e]),
                        pw=_f32(inputs[f"{nm}_pw1"][e]),
                        scale=s1,
                    ),
                    dict(
                        dw=_f32(inputs[f"{nm}_dw2"][e]),
                        pw=_f32(inputs[f"{nm}_pw2"][e]),
                        scale=s2,
                    ),
                ],
                bias1=bb1,
            )
        for k, nm, taps in ((6, "dil3", TAPS3D), (7, "dil5", TAPS5D)):
            c = float(coef[e, k])
            if c == 0.0:
                ops[nm] = None
                continue
            gate = g["gates"][e, k]
            mk = g["mk"][e, k]
            s = c * _f32(inputs[f"{nm}_g"][e]) * gate * mk
            state_bias[tgt] += c * _f32(inputs[f"{nm}_b"][e]) * gate * mk
            if not s.any():
                ops[nm] = None
                continue
            ops[nm] = dict(
                taps=taps,
                layers=[
                    dict(
                        dw=_f32(inputs[f"{nm}_dw"][e]),
                        pw=_f32(inputs[f"{nm}_pw"][e]),
                        scale=s,
                    )
                ],
            )
        if ops["dil3"] is not None and ops["dil5"] is not None:
            # merge dil3 into dil5: every dil3 offset is also a dil5 offset and
            # both accumulate into the same psum -> sum the fused matrices.
            shared = [t for t in TAPS5D if t in TAPS3D]
            excl = [t for t in TAPS5D if t not in TAPS3D]
            lay5 = ops["dil5"]["layers"][0]
            # reorder dil5 dw columns to [shared, excl]
            order = [TAPS5D.index(t) for t in shared + excl]
            dw5 = lay5["dw"].reshape(C, 25)[:, order]
            ops["dil5"] = dict(
                taps=shared + excl,
                layers=[
                    dict(
                        dw=dw5,
                        pw=lay5["pw"],
                        scale=lay5["scale"],
                        merge=ops["dil3"]["layers"][0],
                        merge_taps=shared,
                    )
                ],
                max_donate=len(excl),
            )
            ops["dil3"] = None
        edges.append(dict(e=e, step=i, src=j, tgt=tgt, ops=ops))

    cnt1 = np.full(HH, 3.0, np.float32)
    cnt1[0] = cnt1[-1] = 2.0
    cnt = np.float32(1.0) / np.outer(cnt1, cnt1).astype(np.float32)
    rcnt = np.broadcast_to(cnt.reshape(1, PIX), (C, PIX)).copy()

    plan = dict(
        edges=edges,
        wpre0=wpre0,
        wpre1=wpre1,
        bias0=bias0,
        bias1=bias1,
        state_bias=state_bias,
        rcnt=rcnt,
    )
    _solve_balance(plan)
    _fuse_weights(plan)
    return plan


def _layer_units(plan):
    units = []
    for ed in plan["edges"]:
        for nm in ("sep3", "sep5", "dil3", "dil5"):
            op = ed["ops"][nm]
            if op is None:
                continue
            cap = op.get("max_donate", len(op["taps"]) - 1)
            for li in range(len(op["layers"])):
                units.append((ed, nm, li, op["taps"], cap))
    return units


def _solve_balance(plan):
    """Assign DVE-donated taps. Total donation count D and max chain depth are
    tunable; default D from the analytic balance."""
    units = _layer_units(plan)
    total_taps = sum(len(t) for _, _, _, t, _ in units)

    used_max, used_avg = set(), set()
    n_contrib = 0
    for ed in plan["edges"]:
        if ed["ops"]["max"] != 0.0:
            used_max.add(ed["src"])
            n_contrib += 1
        if ed["ops"]["avg"] != 0.0:
            used_avg.add(ed["src"])
            n_contrib += 1
        if ed["ops"]["skip"] != 0.0:
            n_contrib += 1
    dve_fixed = (
        len(used_max) * (4 * DVE_TT_BIG + 4 * DVE_TT_SMALL)
        + len(used_avg) * (5 * DVE_TT_BIG + 4 * DVE_TT_SMALL)
        + n_contrib * DVE_STT
        + 4 * 1350.0
    )
    pe_fixed = 16 * PE_MM

    D_env = os.environ.get("KERNEL_DVE_TAPS")
    if D_env is not None:
        D_target = int(D_env)
    else:
        # balance: PE = pe_fixed + (total - D)*PE_TAP + 2*PE_MM*nsplit
        # DVE = dve_fixed + (D - nsplit)*DVE_STT ; assume nsplit ~ D/12
        factor = float(os.environ.get("KERNEL_DVE_FACTOR", "1.18"))
        D_target = int(
            max(
                0.0,
                factor
                * (pe_fixed + total_taps * PE_TAP - dve_fixed)
                / (PE_TAP + DVE_STT),
            )
        )
    max_chain = int(os.environ.get("KERNEL_MAX_CHAIN", "12"))

    donors = sorted(range(len(units)), key=lambda i: -units[i][4])
    dve_k = [0] * len(units)
    D = 0
    di = 0
    live = [i for i in donors if units[i][4] > 0]
    while live and D < D_target:
        i = live[di % len(live)]
        cap = min(len(units[i][3]) - 1, units[i][4], max_chain)
        if dve_k[i] >= cap:
            live.remove(i)
            continue
        dve_k[i] += 1
        D += 1
        di += 1
    for i, (ed, nm, li, taps, cap) in enumerate(units):
        ed["ops"][nm].setdefault("dve_k", {})[li] = dve_k[i]
    plan["n_dve_taps"] = D


def _fuse_weights(plan):
    wall_blocks = []
    wall_off = 0
    dw_cols = []

    def add_block(mat):
        nonlocal wall_off
        wall_blocks.append(mat)
        off = wall_off
        wall_off += mat.shape[1] // 128
        return off

    for ed in plan["edges"]:
        for nm in ("sep3", "sep5", "dil3", "dil5"):
            op = ed["ops"][nm]
            if op is None:
                continue
            taps = op["taps"]
            T = len(taps)
            op["emit"] = []
            for li, lay in enumerate(op["layers"]):
                k_dve = op["dve_k"][li]
                pe_taps = taps[: T - k_dve]
                dve_taps = taps[T - k_dve :]
                dwf = lay["dw"].reshape(C, T)
                pws = lay["pw"] * lay["scale"][:, None]  # (O, Cin)
                ent = dict(
                    pe_off=None,
                    pe_taps=pe_taps,
                    dve=[],
                    pw_off=None,
                    wclass=T,
                )
                if pe_taps:
                    idxs = list(range(T - k_dve))
                    blk3 = dwf[:, idxs][:, :, None] * pws.T[:, None, :]  # (C,T',O)
                    if "merge" in lay:
                        mlay = lay["merge"]
                        mdw = mlay["dw"].reshape(C, len(lay["merge_taps"]))
                        mpws = mlay["pw"] * mlay["scale"][:, None]
                        for mt, tap in enumerate(lay["merge_taps"]):
                            ti = pe_taps.index(tap)
                            blk3[:, ti, :] += mdw[:, mt : mt + 1] * mpws.T
                    blk = blk3.reshape(C, len(idxs) * 128)
                    ent["pe_off"] = add_block(blk.astype(np.float32))
                if dve_taps:
                    for t_i, (dy, dx) in enumerate(dve_taps):
                        col = len(dw_cols)
                        dw_cols.append(dwf[:, T - k_dve + t_i])
                        ent["dve"].append((dy, dx, col))
                    ent["pw_off"] = add_block(pws.T.copy().astype(np.float32))
                op["emit"].append(ent)

    plan["wall"] = (
        np.concatenate(wall_blocks, axis=1)
        if wall_blocks
        else np.zeros((C, 128), np.float32)
    )
    plan["dwtab"] = (
        np.stack(dw_cols, axis=1).astype(np.float32)
        if dw_cols
        else np.zeros((C, 1), np.float32)
    )


# ---------------------------------------------------------------------------
# Numpy executor (host model of the device plan; for correctness testing)
# ---------------------------------------------------------------------------


def run_plan_numpy(plan, s0, s1):
    wall = plan["wall"]
    dwtab = plan["dwtab"]

    def pad_img(x, pad, fill=0.0):
        out = np.full((C, HH + 2 * pad, WW + 2 * pad), fill, np.float32)
        out[:, pad : pad + HH, pad : pad + WW] = x
        return out

    def win(xpad, pad, dy, dx):
        return xpad[:, pad + dy : pad + dy + HH, pad + dx : pad + dx + WW].reshape(
            C, PIX
        )

    def layer_out(ent, xpad, pad):
        acc = np.zeros((C, PIX), np.float32)
        for t, (dy, dx) in enumerate(ent["pe_taps"]):
            lhsT = wall[:, (ent["pe_off"] + t) * 128 : (ent["pe_off"] + t + 1) * 128]
            acc += lhsT.T @ win(xpad, pad, dy, dx)
        if ent["dve"]:
            dwacc = np.zeros((C, PIX), np.float32)
            for dy, dx, col in ent["dve"]:
                dwacc += dwtab[:, col : col + 1] * win(xpad, pad, dy, dx)
            lhsT = wall[:, ent["pw_off"] * 128 : (ent["pw_off"] + 1) * 128]
            acc += lhsT.T @ dwacc
        return acc

    states = []
    for s, w, bia in (
        (s0, plan["wpre0"], plan["bias0"]),
        (s1, plan["wpre1"], plan["bias1"]),
    ):
        r = np.maximum(s, 0.0)
        h = w.T @ r + bia[:, None]
        states.append(h.astype(np.float32))

    for i in range(STEPS):
        tgt = 2 + i
        acc = np.zeros((C, PIX), np.float32)
        acc += plan["state_bias"][tgt][:, None]
        for ed in plan["edges"]:
            if ed["step"] != i:
                continue
            x = states[ed["src"]].reshape(C, HH, WW)
            ops = ed["ops"]
            if ops["max"] != 0.0:
                xm = pad_img(x, 1, -np.inf)
                m = np.full((C, HH, WW), -np.inf, np.float32)
                for dy in (-1, 0, 1):
                    for dx in (-1, 0, 1):
                        m = np.maximum(
                            m, xm[:, 1 + dy : 1 + dy + HH, 1 + dx : 1 + dx + WW]
                        )
                acc += ops["max"] * m.reshape(C, PIX)
            if ops["avg"] != 0.0:
                xa = pad_img(x, 1, 0.0)
                ssum = np.zeros((C, HH, WW), np.float32)
                for dy in (-1, 0, 1):
                    for dx in (-1, 0, 1):
                        ssum += xa[:, 1 + dy : 1 + dy + HH, 1 + dx : 1 + dx + WW]
                acc += ops["avg"] * (ssum.reshape(C, PIX) * plan["rcnt"])
            if ops["skip"] != 0.0:
                acc += ops["skip"] * x.reshape(C, PIX)
            rp = pad_img(np.maximum(x, 0.0), 4)
            for nm in ("sep3", "sep5"):
                op = ops[nm]
                if op is None:
                    continue
                mid = layer_out(op["emit"][0], rp, 4)
                mid = np.maximum(mid + op["bias1"][:, None], 0.0)
                mp = pad_img(mid.reshape(C, HH, WW), 2)
                acc += layer_out(op["emit"][1], mp, 2)
            for nm in ("dil3", "dil5"):
                op = ops[nm]
                if op is None:
                    continue
                acc += layer_out(op["emit"][0], rp, 4)
        states.append(acc)

    return np.stack(states[2:], axis=0)


# ---------------------------------------------------------------------------
# Bass device program
# ---------------------------------------------------------------------------


def build_device_program(plan):
    from contextlib import ExitStack

    import concourse.bacc as bacc
    import concourse.mybir as mybir
    import concourse.tile as tile

    F32 = mybir.dt.float32
    F32R = mybir.dt.float32r
    AO = mybir.AluOpType
    AF = mybir.ActivationFunctionType
    AOm, AOa = AO.mult, AO.add

    wall_np = plan["wall"]
    n_wall_taps = wall_np.shape[1] // 128
    n_dw = plan["dwtab"].shape[1]

    nc = bacc.Bacc("TRN2", target_bir_lowering=False, debug=False)
    d_s0 = nc.dram_tensor("s0b", [4, 128, PIX], F32, kind="ExternalInput").ap()
    d_s1 = nc.dram_tensor("s1b", [4, 128, PIX], F32, kind="ExternalInput").ap()
    d_wall = nc.dram_tensor(
        "wall", [128, n_wall_taps * 128], F32R, kind="ExternalInput"
    ).ap()
    d_wpre = nc.dram_tensor("wpre", [128, 1024], F32R, kind="ExternalInput").ap()
    d_btab = nc.dram_tensor("btab", [128, 64], F32, kind="ExternalInput").ap()
    d_dwtab = nc.dram_tensor("dwtab", [128, n_dw], F32, kind="ExternalInput").ap()
    d_rcnt = nc.dram_tensor("rcnt", [128, PIX], F32, kind="ExternalInput").ap()
    d_out = nc.dram_tensor("out", [4, 128, PIX], F32, kind="ExternalOutput").ap()

    bias_cols = {}
    next_bias = 6
    for ed in plan["edges"]:
        for nm in ("sep3", "sep5"):
            if ed["ops"][nm] is not None:
                bias_cols[(ed["e"], nm)] = next_bias
                next_bias += 1
    assert next_bias <= 64

    used_max, used_avg = set(), set()
    for ed in plan["edges"]:
        if ed["ops"]["max"] != 0.0:
            used_max.add(ed["src"])
        if ed["ops"]["avg"] != 0.0:
            used_avg.add(ed["src"])

    with tile.TileContext(nc) as tc, ExitStack() as ctx:
        const = ctx.enter_context(tc.tile_pool(name="const", bufs=1))
        stp = ctx.enter_context(tc.tile_pool(name="stp", bufs=1))
        poolp = ctx.enter_context(tc.tile_pool(name="poolp", bufs=1))
        padp = ctx.enter_context(tc.tile_pool(name="padp", bufs=1))
        extrap = ctx.enter_context(tc.tile_pool(name="extrap", bufs=3))
        psum = ctx.enter_context(tc.tile_pool(name="psum", bufs=2, space="PSUM"))

        btab = const.tile([128, 64], F32, tag="btab", name="btab")
        nc.sync.dma_start(btab[:], d_btab)
        rcnt = const.tile([128, PIX], F32, tag="rcnt", name="rcnt")
        nc.sync.dma_start(rcnt[:], d_rcnt)
        dwtab = const.tile([128, n_dw], F32, tag="dwtab", name="dwtab")
        nc.sync.dma_start(dwtab[:], d_dwtab)

        n_rpad = int(os.environ.get("KERNEL_NRPAD", "3"))
        n_mpad = int(os.environ.get("KERNEL_NMPAD", "4"))
        rpads = []
        for ri in range(n_rpad):
            t = padp.tile([128, 40, 40], F32R, tag=f"rpad{ri}", name=f"rpad{ri}")
            nc.gpsimd.memset(t[:].bitcast(F32), 0.0)
            rpads.append(t)
        mpads = []
        for mi in range(n_mpad):
            t = padp.tile([128, 36, 36], F32R, tag=f"mpad{mi}", name=f"mpad{mi}")
            nc.gpsimd.memset(t[:].bitcast(F32), 0.0)
            mpads.append(t)

        def bias_ap(col):
            return btab[:, col : col + 1]

        # ---- preprocess in its own (released-early) pool
        states = []
        with tc.tile_pool(name="prep", bufs=1) as prep:
            wpre = prep.tile([128, 1024], F32R, tag="wpre", name="wpre")
            nc.sync.dma_start(wpre[:], d_wpre)
            for si, dsrc in enumerate((d_s0, d_s1)):
                rel = {}
                for h in range(2):
                    for kk in range(4):
                        st = prep.tile(
                            [128, 512], F32, tag="stage", name="stage", bufs=4
                        )
                        nc.sync.dma_start(st[:], dsrc[kk][:, 512 * h : 512 * (h + 1)])
                        rl = prep.tile(
                            [128, 512], F32R, tag="relu", name="relu", bufs=10
                        )
                        nc.scalar.activation(rl[:], st[:], AF.Relu)
                        rel[(kk, h)] = rl
                ps = psum.tile([128, PIX], F32, tag="acc", name="pre_acc")
                for h in range(2):
                    for kk in range(4):
                        nc.tensor.matmul(
                            ps[:, 512 * h : 512 * (h + 1)],
                            wpre[:, 512 * si + 128 * kk : 512 * si + 128 * (kk + 1)],
                            rel[(kk, h)][:],
                            start=(kk == 0),
                            stop=(kk == 3),
                        )
                stt = stp.tile(
                    [128, HH, WW], F32, tag=f"state{si}", name=f"state{si}"
                )
                nc.scalar.activation(
                    stt[:],
                    ps[:].rearrange("p (a b) -> p a b", a=HH),
                    AF.Identity,
                    bias=bias_ap(si),
                )
                states.append(stt)

        scratch = ctx.enter_context(tc.tile_pool(name="scratch", bufs=3))
        wp = ctx.enter_context(tc.tile_pool(name="wp", bufs=3))
        dwp = ctx.enter_context(tc.tile_pool(name="dwp", bufs=3))

        maxp_cache = {}
        avgp_cache = {}

        def pool_pass(x, out, tmp, op):
            tt = nc.vector.tensor_tensor
            tt(tmp[:, :, 1:31], x[:, :, 0:30], x[:, :, 1:31], op=op)
            tt(tmp[:, :, 1:31], tmp[:, :, 1:31], x[:, :, 2:32], op=op)
            tt(tmp[:, :, 0:1], x[:, :, 0:1], x[:, :, 1:2], op=op)
            tt(tmp[:, :, 31:32], x[:, :, 30:31], x[:, :, 31:32], op=op)
            tt(out[:, 1:31, :], tmp[:, 0:30, :], tmp[:, 1:31, :], op=op)
            tt(out[:, 1:31, :], out[:, 1:31, :], tmp[:, 2:32, :], op=op)
            tt(out[:, 0:1, :], tmp[:, 0:1, :], tmp[:, 1:2, :], op=op)
            tt(out[:, 31:32, :], tmp[:, 30:31, :], tmp[:, 31:32, :], op=op)

        def get_maxp(s):
            if s not in maxp_cache:
                tmp = scratch.tile([128, HH, WW], F32, tag="ptmp", name="ptmp", bufs=2)
                out = poolp.tile([128, HH, WW], F32, tag=f"maxp{s}", name=f"maxp{s}")
                pool_pass(states[s], out, tmp, mybir.AluOpType.max)
                maxp_cache[s] = out
            return maxp_cache[s]

        def get_avgp(s):
            if s not in avgp_cache:
                tmp = scratch.tile([128, HH, WW], F32, tag="ptmp", name="ptmp", bufs=2)
                out = poolp.tile([128, HH, WW], F32, tag=f"avgp{s}", name=f"avgp{s}")
                pool_pass(states[s], out, tmp, mybir.AluOpType.add)
                nc.vector.tensor_tensor(
                    out[:].rearrange("p a b -> p (a b)"),
                    out[:].rearrange("p a b -> p (a b)"),
                    rcnt[:],
                    op=mybir.AluOpType.mult,
                )
                avgp_cache[s] = out
            return avgp_cache[s]

        mpad_rot = [0]

        for i in range(STEPS):
            tgt = 2 + i
            step_edges = [ed for ed in plan["edges"] if ed["step"] == i]
            n_acc = 0
            any_extra = False
            for ed in step_edges:
                for nm in ("sep3", "sep5", "dil3", "dil5"):
                    op = ed["ops"][nm]
                    if op is None:
                        continue
                    ent = op["emit"][-1]
                    n_acc += len(ent["pe_taps"]) + (1 if ent["dve"] else 0)
                if (
                    ed["ops"]["max"] != 0.0
                    or ed["ops"]["avg"] != 0.0
                    or ed["ops"]["skip"] != 0.0
                ):
                    any_extra = True

            acc = psum.tile([128, PIX], F32, tag="acc", name="acc") if n_acc else None
            extra = None
            if any_extra:
                extra = extrap.tile([128, PIX], F32, tag="extra", name="extra")
                nc.gpsimd.memset(extra[:], 0.0)
            acc_idx = [0, 0]

            def acc_mm(h, lhsT, rhs):
                nc.tensor.matmul(
                    acc[:, 512 * h : 512 * (h + 1)],
                    lhsT,
                    rhs,
                    start=(acc_idx[h] == 0),
                    stop=(acc_idx[h] == n_acc - 1),
                )
                acc_idx[h] += 1

            def dma_weights(ent):
                tiles = {}
                if ent["pe_taps"]:
                    ntap = len(ent["pe_taps"])
                    wt = wp.tile(
                        [128, ntap * 128],
                        F32R,
                        tag=f"w{ent['wclass']}",
                        name="wt",
                        bufs=int(os.environ.get("KERNEL_WBUFS", "4")),
                    )
                    nc.sync.dma_start(
                        wt[:],
                        d_wall[:, ent["pe_off"] * 128 : (ent["pe_off"] + ntap) * 128],
                    )
                    tiles["pe"] = wt
                if ent["dve"]:
                    wt = wp.tile([128, 128], F32R, tag="wpw", name="wtp", bufs=int(os.environ.get("KERNEL_PWBUFS", "8")))
                    nc.sync.dma_start(
                        wt[:],
                        d_wall[:, ent["pw_off"] * 128 : (ent["pw_off"] + 1) * 128],
                    )
                    tiles["pw"] = wt
                return tiles

            def emit_chain(ent, src_t, pad):
                if not ent["dve"]:
                    return
                dwacc = dwp.tile(
                    [128, HH, WW],
                    F32R,
                    tag="dwacc",
                    name="dwacc",
                    bufs=int(os.environ.get("KERNEL_DWBUFS", "4")),
                )
                for t_i, (dy, dx, col) in enumerate(ent["dve"]):
                    w3 = src_t[:, pad + dy : pad + dy + HH, pad + dx : pad + dx + WW]
                    if t_i == 0:
                        nc.scalar.activation(
                            dwacc[:], w3, AF.Copy, scale=dwtab[:, col : col + 1]
                        )
                    else:
                        nc.vector.scalar_tensor_tensor(
                            dwacc[:],
                            w3,
                            dwtab[:, col : col + 1],
                            dwacc[:],
                            op0=AOm,
                            op1=AOa,
                        )
                ent["_dwacc"] = dwacc

            def emit_layer(ent, tiles, src_t, pad, into_acc, mid_ps=None):
                n_mm = 2 * (len(ent["pe_taps"]) + (1 if ent["dve"] else 0))
                mm_i = [0]

                def do_mm(lhsT, rhs, h):
                    if into_acc:
                        acc_mm(h, lhsT, rhs)
                    else:
                        nc.tensor.matmul(
                            mid_ps[h][:],
                            lhsT,
                            rhs,
                            start=(mm_i[0] < 2),
                            stop=(mm_i[0] >= n_mm - 2),
                        )
                    mm_i[0] += 1

                dwacc = ent.pop("_dwacc", None)
                for t, (dy, dx) in enumerate(ent["pe_taps"]):
                    lhsT = tiles["pe"][:, t * 128 : (t + 1) * 128]
                    for h in range(2):
                        rhs = src_t[
                            :,
                            pad + dy + 16 * h : pad + dy + 16 * h + 16,
                            pad + dx : pad + dx + 32,
                        ]
                        do_mm(lhsT, rhs, h)
                if ent["dve"]:
                    df = dwacc[:].rearrange("p a b -> p (a b)")
                    for h in range(2):
                        do_mm(tiles["pw"], df[:, 512 * h : 512 * (h + 1)], h)

            for ed in step_edges:
                s = ed["src"]
                ops = ed["ops"]
                x = states[s]
                xf = x[:].rearrange("p a b -> p (a b)")
                stt_op = nc.vector.scalar_tensor_tensor

                live = [
                    nm for nm in ("sep3", "sep5", "dil3", "dil5") if ops[nm] is not None
                ]
                if live:
                    rp = rpads[ed["e"] % n_rpad]
                    nc.scalar.activation(rp[:, 4:36, 4:36], x[:], AF.Relu)
                    tiles = {
                        nm: [dma_weights(ent) for ent in ops[nm]["emit"]]
                        for nm in live
                    }
                    # DVE dw chains for rpad-fed layers first (high priority)
                    for nm in live:
                        emit_chain(ops[nm]["emit"][0], rp, 4)
                    # PE taps: dil layers straight into acc
                    for nm in ("dil3", "dil5"):
                        if nm in live:
                            emit_layer(ops[nm]["emit"][0], tiles[nm][0], rp, 4, True)
                    # sep layers: L1 -> mpad -> (L2 chain) -> L2 taps
                    for nm in ("sep3", "sep5"):
                        if nm not in live:
                            continue
                        op = ops[nm]
                        ent1, ent2 = op["emit"]
                        mid = [
                            psum.tile([128, 512], F32, tag="mid", name="mid", bufs=4)
                            for _ in range(2)
                        ]
                        emit_layer(ent1, tiles[nm][0], rp, 4, False, mid)
                        mpad = mpads[mpad_rot[0] % n_mpad]
                        mpad_rot[0] += 1
                        for h in range(2):
                            nc.scalar.activation(
                                mpad[:, 2 + 16 * h : 18 + 16 * h, 2:34],
                                mid[h][:].rearrange("p (a b) -> p a b", a=16),
                                AF.Relu,
                                bias=bias_ap(bias_cols[(ed["e"], nm)]),
                            )
                        emit_chain(ent2, mpad, 2)
                        emit_layer(ent2, tiles[nm][1], mpad, 2, True)

                # pool/skip contributions (not on the PE critical path)
                if ops["max"] != 0.0:
                    mp = get_maxp(s)
                    stt_op(
                        extra[:],
                        mp[:].rearrange("p a b -> p (a b)"),
                        ops["max"],
                        extra[:],
                        op0=AOm,
                        op1=AOa,
                    )
                if ops["avg"] != 0.0:
                    ap_ = get_avgp(s)
                    stt_op(
                        extra[:],
                        ap_[:].rearrange("p a b -> p (a b)"),
                        ops["avg"],
                        extra[:],
                        op0=AOm,
                        op1=AOa,
                    )
                if ops["skip"] != 0.0:
                    stt_op(extra[:], xf, ops["skip"], extra[:], op0=AOm, op1=AOa)

            assert acc_idx[0] == n_acc and acc_idx[1] == n_acc, (acc_idx, n_acc)

            stt = stp.tile([128, HH, WW], F32, tag=f"state{tgt}", name=f"state{tgt}")
            sb = bias_ap(2 + (tgt - 2))
            if acc is not None and extra is not None:
                sf = stt[:].rearrange("p a b -> p (a b)")
                for h in range(2):
                    nc.vector.scalar_tensor_tensor(
                        sf[:, 512 * h : 512 * (h + 1)],
                        acc[:, 512 * h : 512 * (h + 1)],
                        sb,
                        extra[:, 512 * h : 512 * (h + 1)],
                        op0=AOa,
                        op1=AOa,
                    )
            elif acc is not None:
                nc.scalar.activation(
                    stt[:],
                    acc[:].rearrange("p (a b) -> p a b", a=HH),
                    AF.Identity,
                    bias=sb,
                )
            elif extra is not None:
                nc.vector.tensor_scalar(
                    stt[:].rearrange("p a b -> p (a b)"), extra[:], sb, None, op0=AOa
                )
            else:
                nc.vector.memset(stt[:], 0.0)
            states.append(stt)

            so = stt[:].rearrange("p a b -> p (a b)")
            for h in range(2):
                nc.sync.dma_start(
                    d_out[i][:, 512 * h : 512 * (h + 1)], so[:, 512 * h : 512 * (h + 1)]
                )

    nc.compile()
    return nc


def _make_btab(plan):
    btab = np.zeros((128, 64), np.float32)
    btab[:, 0] = plan["bias0"]
    btab[:, 1] = plan["bias1"]
    for i in range(4):
        btab[:, 2 + i] = plan["state_bias"][2 + i]
    col = 6
    for ed in plan["edges"]:
        for nm in ("sep3", "sep5"):
            if ed["ops"][nm] is not None:
                btab[:, col] = ed["ops"][nm]["bias1"]
                col += 1
    return btab


def make_in_maps(plan, inputs):
    wpre = np.zeros((128, 1024), np.float32)
    wpre[:, 0:512] = (
        plan["wpre0"].reshape(4, 128, 128).transpose(1, 0, 2).reshape(128, 512)
    )
    wpre[:, 512:1024] = (
        plan["wpre1"].reshape(4, 128, 128).transpose(1, 0, 2).reshape(128, 512)
    )
    btab = _make_btab(plan)
    s0 = _f32(inputs["s0"]).reshape(B, 4, 128, PIX)
    s1 = _f32(inputs["s1"]).reshape(B, 4, 128, PIX)
    base = {
        "wall": np.ascontiguousarray(plan["wall"]),
        "wpre": wpre,
        "btab": btab,
        "dwtab": np.ascontiguousarray(plan["dwtab"]),
        "rcnt": plan["rcnt"],
    }
    return [
        {
            **base,
            "s0b": np.ascontiguousarray(s0[b]),
            "s1b": np.ascontiguousarray(s1[b]),
        }
        for b in range(B)
    ]


def kernel(**inputs):
    plan = build_plan(inputs)

    if os.environ.get("KERNEL_NUMPY") == "1":
        s0 = _f32(inputs["s0"]).reshape(B, C_PREV, PIX)
        s1 = _f32(inputs["s1"]).reshape(B, C_PREV, PIX)
        outs = []
        for b in range(B):
            r = run_plan_numpy(plan, s0[b], s1[b])
            outs.append(r.reshape(4 * C, HH, WW))
        return np.stack(outs).astype(np.float32)

    from concourse.bass_utils import run_bass_kernel_spmd

    nc = build_device_program(plan)
    in_maps = make_in_maps(plan, inputs)
    res = run_bass_kernel_spmd(nc, in_maps, core_ids=list(range(N_CORES)))
    out = np.stack([res.results[b]["out"].reshape(4 * C, HH, WW) for b in range(B)])
    return out.astype(np.float32)


# revision 21
# speedup vs baseline: 370.8110x; 370.8110x over previous
"""Trainium2 Bass kernel for nn_Cell_67894843015282 (DARTS-style NAS cell).

Strategy:
  - All routing/gating logic (masks, node_sum chain, sigmoid gates) depends only
    on small parameter tensors -> computed on host in f32 (jax-on-CPU to match
    the reference bit-for-bit on the step() comparisons).
  - BN affine + channel gates + op coefficients folded into the pointwise conv
    matrices (per-output-channel scale) and hoisted bias vectors; ops with a
    zero coefficient are skipped entirely.
  - Depthwise (x) pointwise conv pairs fused into per-tap 128x128 matmuls on the
    tensor engine (fp32r), accumulating across taps/ops/edges directly in PSUM.
    A balance solver spills some taps to the vector engine (per-partition-scalar
    multiply-accumulate chains) with the pointwise applied by one extra matmul.
  - Pools (max/avg 3x3) computed once per source state on the vector engine
    (separable passes); skip/pool contributions accumulated with
    scalar_tensor_tensor into an SBUF accumulator.
  - Data parallel over batch: 1 image per NeuronCore, 8 cores.
"""

import os

import numpy as np

B, C, HH, WW = 8, 128, 32, 32
PIX = HH * WW
C_PREV = 512
STEPS, N_EDGES, N_OPS = 4, 14, 8
N_CORES = 8

# cost-model constants (ns) for the balance solver
PE_TAP = 470.0  # two N=512 fp32r matmuls
PE_MM = 235.0
DVE_STT = 1222.0  # scalar_tensor_tensor on (128,1024)
DVE_TS = 689.0  # tensor_scalar f32 (single-src, 2x) -- unused, tap0 is on ACT
DVE_TT_BIG = 1222.0
DVE_TT_SMALL = 250.0
ACT_OP = 1257.0

# ---------------------------------------------------------------------------
# Host-side gating / fusion (the "plan")
# ---------------------------------------------------------------------------


def _f32(x):
    return np.asarray(x, dtype=np.float32)


def _gate_math(inputs):
    """Replicate the data-independent gating chain of the reference in f32.

    Uses jax on CPU when available so the step() threshold comparisons match
    the reference numerics exactly; falls back to numpy.
    """
    try:
        import jax

        cpu = jax.devices("cpu")[0]

        with jax.default_device(cpu):
            import jax.numpy as jnp

            return _gate_math_impl(jnp, jax.nn.sigmoid, inputs, to_np=np.asarray)
    except Exception:

        def np_sig(x):
            return 1.0 / (1.0 + np.exp(-np.asarray(x, np.float32), dtype=np.float32))

        return _gate_math_impl(np, np_sig, inputs, to_np=np.asarray)


def _gate_math_impl(xp, sig, inputs, to_np):
    f32 = np.float32
    weights2 = xp.asarray(inputs["weights2"], dtype=f32)
    thre = xp.asarray(inputs["thre"], dtype=f32)
    mask_default = xp.asarray(inputs["mask_default"])
    kernel_param = xp.asarray(inputs["kernel_param"], dtype=f32)
    mask_k_default = xp.asarray(inputs["mask_k_default"])
    mask_w_default = xp.asarray(inputs["mask_w_default"])
    kernel_pre = xp.asarray(inputs["kernel_pre"], dtype=f32)
    thre_pre = xp.asarray(inputs["thre_pre"], dtype=f32)

    def step(x):
        return (x > 0).astype(f32)

    mdf = mask_default.astype(f32)

    g0 = sig(kernel_pre[0])
    mk0 = step(g0 - thre_pre[0])
    gv0 = to_np(g0 * mk0).astype(f32)
    g1p = sig(kernel_pre[1])
    mk1 = step(g1p - thre_pre[1])
    gv1 = to_np(g1p * mk1).astype(f32)

    n_states = 2
    offset = 0
    m_all = np.zeros((N_EDGES, N_OPS), np.float32)
    for i in range(STEPS):
        n = n_states
        weight_sum = (weights2[offset : offset + n] * mdf[offset : offset + n]).sum()
        for j in range(n):
            e = offset + j
            ns = weight_sum
            m_list = []
            for k in range(N_OPS):
                w = weights2[e, k]
                md = mdf[e, k]
                m = xp.where(
                    md == 0, f32(0.0), xp.where(w != ns, step(w - thre[e, k, 0]), md)
                )
                cond = (md != 0) & (w != ns) & (m == 0)
                m_list.append(m)
                ns = xp.where(cond, ns - w, ns)
            m_vec = xp.stack(m_list)
            weight_sum = (
                weight_sum - (weights2[e] * mdf[e]).sum() + (weights2[e] * m_vec).sum()
            )
            m_all[e] = to_np(m_vec)
        offset += n
        n_states += 1

    coef = to_np(weights2).astype(f32) * m_all

    gates = to_np(sig(kernel_param)).astype(f32)
    t1 = to_np(thre[:, :, 1]).astype(f32)
    t2 = to_np(thre[:, :, 2]).astype(f32)
    mk = (gates - t1[:, :, None] > 0).astype(f32) * (to_np(mask_k_default) != 0)
    mw = (gates - t2[:, :, None] > 0).astype(f32) * (to_np(mask_w_default) != 0)
    return dict(
        gv0=gv0,
        gv1=gv1,
        coef=coef,
        gates=gates,
        mk=mk.astype(np.float32),
        mw=mw.astype(np.float32),
    )


TAPS3 = [(dy, dx) for dy in (-1, 0, 1) for dx in (-1, 0, 1)]
TAPS5 = [(dy, dx) for dy in (-2, -1, 0, 1, 2) for dx in (-2, -1, 0, 1, 2)]
TAPS3D = [(dy, dx) for dy in (-2, 0, 2) for dx in (-2, 0, 2)]
TAPS5D = [(dy, dx) for dy in (-4, -2, 0, 2, 4) for dx in (-4, -2, 0, 2, 4)]


def build_plan(inputs):
    g = _gate_math(inputs)
    coef = g["coef"]

    scale0 = _f32(inputs["pre0_g"]) * g["gv0"]
    bias0 = _f32(inputs["pre0_b"]) * g["gv0"]
    scale1 = _f32(inputs["pre1_g"]) * g["gv1"]
    bias1 = _f32(inputs["pre1_b"]) * g["gv1"]
    wpre0 = (_f32(inputs["pre0_w"]) * scale0[:, None]).T.copy()  # (512,128)
    wpre1 = (_f32(inputs["pre1_w"]) * scale1[:, None]).T.copy()

    state_of_edge = []
    for i in range(STEPS):
        for j in range(2 + i):
            state_of_edge.append((i, j))

    edges = []
    state_bias = np.zeros((6, C), np.float32)
    for e in range(N_EDGES):
        i, j = state_of_edge[e]
        tgt = 2 + i
        ops = {
            "max": float(coef[e, 1]),
            "avg": float(coef[e, 2]),
            "skip": float(coef[e, 3]),
        }
        for k, nm, taps in ((4, "sep3", TAPS3), (5, "sep5", TAPS5)):
            c = float(coef[e, k])
            if c == 0.0:
                ops[nm] = None
                continue
            gate = g["gates"][e, k]
            mk = g["mk"][e, k]
            mw = g["mw"][e, k]
            s1 = _f32(inputs[f"{nm}_g1"][e]) * gate * mk
            bb1 = _f32(inputs[f"{nm}_b1"][e]) * gate * mk
            s2 = c * _f32(inputs[f"{nm}_g2"][e]) * gate * mw
            bb2 = c * _f32(inputs[f"{nm}_b2"][e]) * gate * mw
            state_bias[tgt] += bb2
            if not s2.any() or not (s1.any() or bb1.any()):
                ops[nm] = None
                continue
            ops[nm] = dict(
                taps=taps,
                layers=[
                    dict(
                        dw=_f32(inputs[f"{nm}_dw1"][e]),
                        pw=_f32(inputs[f"{nm}_pw1"][e]),
                        scale=s1,
                    ),
                    dict(
                        dw=_f32(inputs[f"{nm}_dw2"][e]),
                        pw=_f32(inputs[f"{nm}_pw2"][e]),
                        scale=s2,
                    ),
                ],
                bias1=bb1,
            )
        for k, nm, taps in ((6, "dil3", TAPS3D), (7, "dil5", TAPS5D)):
            c = float(coef[e, k])
            if c == 0.0:
                ops[nm] = None
                continue
            gate = g["gates"][e, k]
            mk = g["mk"][e, k]
            s = c * _f32(inputs[f"{nm}_g"][e]) * gate * mk
            state_bias[tgt] += c * _f32(inputs[f"{nm}_b"][e]) * gate * mk
            if not s.any():
                ops[nm] = None
                continue
            ops[nm] = dict(
                taps=taps,
                layers=[
                    dict(
                        dw=_f32(inputs[f"{nm}_dw"][e]),
                        pw=_f32(inputs[f"{nm}_pw"][e]),
                        scale=s,
                    )
                ],
            )
        if ops["dil3"] is not None and ops["dil5"] is not None:
            # merge dil3 into dil5: every dil3 offset is also a dil5 offset and
            # both accumulate into the same psum -> sum the fused matrices.
            shared = [t for t in TAPS5D if t in TAPS3D]
            excl = [t for t in TAPS5D if t not in TAPS3D]
            lay5 = ops["dil5"]["layers"][0]
            # reorder dil5 dw columns to [shared, excl]
            order = [TAPS5D.index(t) for t in shared + excl]
            dw5 = lay5["dw"].reshape(C, 25)[:, order]
            ops["dil5"] = dict(
                taps=shared + excl,
                layers=[
                    dict(
                        dw=dw5,
                        pw=lay5["pw"],
                        scale=lay5["scale"],
                        merge=ops["dil3"]["layers"][0],
                        merge_taps=shared,
                    )
                ],
                max_donate=len(excl),
            )
            ops["dil3"] = None
        edges.append(dict(e=e, step=i, src=j, tgt=tgt, ops=ops))

    cnt1 = np.full(HH, 3.0, np.float32)
    cnt1[0] = cnt1[-1] = 2.0
    cnt = np.float32(1.0) / np.outer(cnt1, cnt1).astype(np.float32)
    rcnt = np.broadcast_to(cnt.reshape(1, PIX), (C, PIX)).copy()

    plan = dict(
        edges=edges,
        wpre0=wpre0,
        wpre1=wpre1,
        bias0=bias0,
        bias1=bias1,
        state_bias=state_bias,
        rcnt=rcnt,
    )
    _solve_balance(plan)
    _fuse_weights(plan)
    return plan


def _layer_units(plan):
    units = []
    for ed in plan["edges"]:
        for nm in ("sep3", "sep5", "dil3", "dil5"):
            op = ed["ops"][nm]
            if op is None:
                continue
            cap = op.get("max_donate", len(op["taps"]) - 1)
            for li in range(len(op["layers"])):
                units.append((ed, nm, li, op["taps"], cap))
    return units


def _solve_balance(plan):
    """Assign DVE-donated taps. Total donation count D and max chain depth are
    tunable; default D from the analytic balance."""
    units = _layer_units(plan)
    total_taps = sum(len(t) for _, _, _, t, _ in units)

    used_max, used_avg = set(), set()
    n_contrib = 0
    for ed in plan["edges"]:
        if ed["ops"]["max"] != 0.0:
            used_max.add(ed["src"])
            n_contrib += 1
        if ed["ops"]["avg"] != 0.0:
            used_avg.add(ed["src"])
            n_contrib += 1
        if ed["ops"]["skip"] != 0.0:
            n_contrib += 1
    dve_fixed = (
        len(used_max) * (4 * DVE_TT_BIG + 4 * DVE_TT_SMALL)
        + len(used_avg) * (5 * DVE_TT_BIG + 4 * DVE_TT_SMALL)
        + n_contrib * DVE_STT
        + 4 * 1350.0
    )
    pe_fixed = 16 * PE_MM

    D_env = os.environ.get("KERNEL_DVE_TAPS")
    if D_env is not None:
        D_target = int(D_env)
    else:
        # balance: PE = pe_fixed + (total - D)*PE_TAP + 2*PE_MM*nsplit
        # DVE = dve_fixed + (D - nsplit)*DVE_STT ; assume nsplit ~ D/12
        factor = float(os.environ.get("KERNEL_DVE_FACTOR", "1.18"))
        D_target = int(
            max(
                0.0,
                factor
                * (pe_fixed + total_taps * PE_TAP - dve_fixed)
                / (PE_TAP + DVE_STT),
            )
        )
    max_chain = int(os.environ.get("KERNEL_MAX_CHAIN", "12"))

    donors = sorted(range(len(units)), key=lambda i: -units[i][4])
    dve_k = [0] * len(units)
    D = 0
    di = 0
    live = [i for i in donors if units[i][4] > 0]
    fill_mode = os.environ.get("KERNEL_DONATE", "rr") == "fill"
    while live and D < D_target:
        i = live[di % len(live)] if not fill_mode else live[0]
        cap = min(len(units[i][3]) - 1, units[i][4], max_chain)
        if dve_k[i] >= cap:
            live.remove(i)
            continue
        dve_k[i] += 1
        D += 1
        if not fill_mode:
            di += 1
    for i, (ed, nm, li, taps, cap) in enumerate(units):
        ed["ops"][nm].setdefault("dve_k", {})[li] = dve_k[i]
    plan["n_dve_taps"] = D


def _fuse_weights(plan):
    wall_blocks = []
    wall_off = 0
    dw_cols = []

    def add_block(mat):
        nonlocal wall_off
        wall_blocks.append(mat)
        off = wall_off
        wall_off += mat.shape[1] // 128
        return off

    for ed in plan["edges"]:
        for nm in ("sep3", "sep5", "dil3", "dil5"):
            op = ed["ops"][nm]
            if op is None:
                continue
            taps = op["taps"]
            T = len(taps)
            op["emit"] = []
            for li, lay in enumerate(op["layers"]):
                k_dve = op["dve_k"][li]
                pe_taps = taps[: T - k_dve]
                dve_taps = taps[T - k_dve :]
                dwf = lay["dw"].reshape(C, T)
                pws = lay["pw"] * lay["scale"][:, None]  # (O, Cin)
                ent = dict(
                    pe_off=None,
                    pe_taps=pe_taps,
                    dve=[],
                    pw_off=None,
                    wclass=T,
                )
                if pe_taps:
                    idxs = list(range(T - k_dve))
                    blk3 = dwf[:, idxs][:, :, None] * pws.T[:, None, :]  # (C,T',O)
                    if "merge" in lay:
                        mlay = lay["merge"]
                        mdw = mlay["dw"].reshape(C, len(lay["merge_taps"]))
                        mpws = mlay["pw"] * mlay["scale"][:, None]
                        for mt, tap in enumerate(lay["merge_taps"]):
                            ti = pe_taps.index(tap)
                            blk3[:, ti, :] += mdw[:, mt : mt + 1] * mpws.T
                    blk = blk3.reshape(C, len(idxs) * 128)
                    ent["pe_off"] = add_block(blk.astype(np.float32))
                if dve_taps:
                    for t_i, (dy, dx) in enumerate(dve_taps):
                        col = len(dw_cols)
                        dw_cols.append(dwf[:, T - k_dve + t_i])
                        ent["dve"].append((dy, dx, col))
                    ent["pw_off"] = add_block(pws.T.copy().astype(np.float32))
                op["emit"].append(ent)

    plan["wall"] = (
        np.concatenate(wall_blocks, axis=1)
        if wall_blocks
        else np.zeros((C, 128), np.float32)
    )
    plan["dwtab"] = (
        np.stack(dw_cols, axis=1).astype(np.float32)
        if dw_cols
        else np.zeros((C, 1), np.float32)
    )


# ---------------------------------------------------------------------------
# Numpy executor (host model of the device plan; for correctness testing)
# ---------------------------------------------------------------------------


def run_plan_numpy(plan, s0, s1):
    wall = plan["wall"]
    dwtab = plan["dwtab"]

    def pad_img(x, pad, fill=0.0):
        out = np.full((C, HH + 2 * pad, WW + 2 * pad), fill, np.float32)
        out[:, pad : pad + HH, pad : pad + WW] = x
        return out

    def win(xpad, pad, dy, dx):
        return xpad[:, pad + dy : pad + dy + HH, pad + dx : pad + dx + WW].reshape(
            C, PIX
        )

    def layer_out(ent, xpad, pad):
        acc = np.zeros((C, PIX), np.float32)
        for t, (dy, dx) in enumerate(ent["pe_taps"]):
            lhsT = wall[:, (ent["pe_off"] + t) * 128 : (ent["pe_off"] + t + 1) * 128]
            acc += lhsT.T @ win(xpad, pad, dy, dx)
        if ent["dve"]:
            dwacc = np.zeros((C, PIX), np.float32)
            for dy, dx, col in ent["dve"]:
                dwacc += dwtab[:, col : col + 1] * win(xpad, pad, dy, dx)
            lhsT = wall[:, ent["pw_off"] * 128 : (ent["pw_off"] + 1) * 128]
            acc += lhsT.T @ dwacc
        return acc

    states = []
    for s, w, bia in (
        (s0, plan["wpre0"], plan["bias0"]),
        (s1, plan["wpre1"], plan["bias1"]),
    ):
        r = np.maximum(s, 0.0)
        h = w.T @ r + bia[:, None]
        states.append(h.astype(np.float32))

    for i in range(STEPS):
        tgt = 2 + i
        acc = np.zeros((C, PIX), np.float32)
        acc += plan["state_bias"][tgt][:, None]
        for ed in plan["edges"]:
            if ed["step"] != i:
                continue
            x = states[ed["src"]].reshape(C, HH, WW)
            ops = ed["ops"]
            if ops["max"] != 0.0:
                xm = pad_img(x, 1, -np.inf)
                m = np.full((C, HH, WW), -np.inf, np.float32)
                for dy in (-1, 0, 1):
                    for dx in (-1, 0, 1):
                        m = np.maximum(
                            m, xm[:, 1 + dy : 1 + dy + HH, 1 + dx : 1 + dx + WW]
                        )
                acc += ops["max"] * m.reshape(C, PIX)
            if ops["avg"] != 0.0:
                xa = pad_img(x, 1, 0.0)
                ssum = np.zeros((C, HH, WW), np.float32)
                for dy in (-1, 0, 1):
                    for dx in (-1, 0, 1):
                        ssum += xa[:, 1 + dy : 1 + dy + HH, 1 + dx : 1 + dx + WW]
                acc += ops["avg"] * (ssum.reshape(C, PIX) * plan["rcnt"])
            if ops["skip"] != 0.0:
                acc += ops["skip"] * x.reshape(C, PIX)
            rp = pad_img(np.maximum(x, 0.0), 4)
            for nm in ("sep3", "sep5"):
                op = ops[nm]
                if op is None:
                    continue
                mid = layer_out(op["emit"][0], rp, 4)
                mid = np.maximum(mid + op["bias1"][:, None], 0.0)
                mp = pad_img(mid.reshape(C, HH, WW), 2)
                acc += layer_out(op["emit"][1], mp, 2)
            for nm in ("dil3", "dil5"):
                op = ops[nm]
                if op is None:
                    continue
                acc += layer_out(op["emit"][0], rp, 4)
        states.append(acc)

    return np.stack(states[2:], axis=0)


# ---------------------------------------------------------------------------
# Bass device program
# ---------------------------------------------------------------------------


def build_device_program(plan):
    from contextlib import ExitStack

    import concourse.bacc as bacc
    import concourse.mybir as mybir
    import concourse.tile as tile

    F32 = mybir.dt.float32
    F32R = mybir.dt.float32r
    AO = mybir.AluOpType
    AF = mybir.ActivationFunctionType
    AOm, AOa = AO.mult, AO.add

    wall_np = plan["wall"]
    n_wall_taps = wall_np.shape[1] // 128
    n_dw = plan["dwtab"].shape[1]

    nc = bacc.Bacc("TRN2", target_bir_lowering=False, debug=False)
    d_s0 = nc.dram_tensor("s0b", [4, 128, PIX], F32, kind="ExternalInput").ap()
    d_s1 = nc.dram_tensor("s1b", [4, 128, PIX], F32, kind="ExternalInput").ap()
    d_wall = nc.dram_tensor(
        "wall", [128, n_wall_taps * 128], F32R, kind="ExternalInput"
    ).ap()
    d_wpre = nc.dram_tensor("wpre", [128, 1024], F32R, kind="ExternalInput").ap()
    d_btab = nc.dram_tensor("btab", [128, 64], F32, kind="ExternalInput").ap()
    d_dwtab = nc.dram_tensor("dwtab", [128, n_dw], F32, kind="ExternalInput").ap()
    d_rcnt = nc.dram_tensor("rcnt", [128, PIX], F32, kind="ExternalInput").ap()
    d_out = nc.dram_tensor("out", [4, 128, PIX], F32, kind="ExternalOutput").ap()

    bias_cols = {}
    next_bias = 6
    for ed in plan["edges"]:
        for nm in ("sep3", "sep5"):
            if ed["ops"][nm] is not None:
                bias_cols[(ed["e"], nm)] = next_bias
                next_bias += 1
    assert next_bias <= 64

    used_max, used_avg = set(), set()
    for ed in plan["edges"]:
        if ed["ops"]["max"] != 0.0:
            used_max.add(ed["src"])
        if ed["ops"]["avg"] != 0.0:
            used_avg.add(ed["src"])

    with tile.TileContext(nc) as tc, ExitStack() as ctx:
        const = ctx.enter_context(tc.tile_pool(name="const", bufs=1))
        stp = ctx.enter_context(tc.tile_pool(name="stp", bufs=1))
        poolp = ctx.enter_context(tc.tile_pool(name="poolp", bufs=1))
        padp = ctx.enter_context(tc.tile_pool(name="padp", bufs=1))
        extrap = ctx.enter_context(tc.tile_pool(name="extrap", bufs=3))
        psum = ctx.enter_context(tc.tile_pool(name="psum", bufs=2, space="PSUM"))

        btab = const.tile([128, 64], F32, tag="btab", name="btab")
        nc.sync.dma_start(btab[:], d_btab)
        rcnt = const.tile([128, PIX], F32, tag="rcnt", name="rcnt")
        nc.sync.dma_start(rcnt[:], d_rcnt)
        dwtab = const.tile([128, n_dw], F32, tag="dwtab", name="dwtab")
        nc.sync.dma_start(dwtab[:], d_dwtab)

        n_rpad = int(os.environ.get("KERNEL_NRPAD", "3"))
        n_mpad = int(os.environ.get("KERNEL_NMPAD", "4"))
        rpads = []
        for ri in range(n_rpad):
            t = padp.tile([128, 40, 40], F32R, tag=f"rpad{ri}", name=f"rpad{ri}")
            nc.gpsimd.memset(t[:].bitcast(F32), 0.0)
            rpads.append(t)

        mpads = []
        for mi in range(n_mpad):
            t = padp.tile([128, 36, 36], F32R, tag=f"mpad{mi}", name=f"mpad{mi}")
            nc.gpsimd.memset(t[:].bitcast(F32), 0.0)
            mpads.append(t)

        def bias_ap(col):
            return btab[:, col : col + 1]

        # ---- preprocess in its own (released-early) pool
        states = []
        with tc.tile_pool(name="prep", bufs=1) as prep:
            wpre = prep.tile([128, 1024], F32R, tag="wpre", name="wpre")
            nc.sync.dma_start(wpre[:], d_wpre)
            for si, dsrc in enumerate((d_s0, d_s1)):
                rel = {}
                for h in range(2):
                    for kk in range(4):
                        st = prep.tile(
                            [128, 512], F32, tag="stage", name="stage", bufs=4
                        )
                        nc.sync.dma_start(st[:], dsrc[kk][:, 512 * h : 512 * (h + 1)])
                        rl = prep.tile(
                            [128, 512], F32R, tag="relu", name="relu", bufs=10
                        )
                        nc.scalar.activation(rl[:], st[:], AF.Relu)
                        rel[(kk, h)] = rl
                ps = psum.tile([128, PIX], F32, tag="acc", name="pre_acc")
                for h in range(2):
                    for kk in range(4):
                        nc.tensor.matmul(
                            ps[:, 512 * h : 512 * (h + 1)],
                            wpre[:, 512 * si + 128 * kk : 512 * si + 128 * (kk + 1)],
                            rel[(kk, h)][:],
                            start=(kk == 0),
                            stop=(kk == 3),
                        )
                stt = stp.tile(
                    [128, HH, WW], F32, tag=f"state{si}", name=f"state{si}"
                )
                nc.scalar.activation(
                    stt[:],
                    ps[:].rearrange("p (a b) -> p a b", a=HH),
                    AF.Identity,
                    bias=bias_ap(si),
                )
                states.append(stt)

        scratch = ctx.enter_context(tc.tile_pool(name="scratch", bufs=3))
        wp = ctx.enter_context(tc.tile_pool(name="wp", bufs=3))
        dwp = ctx.enter_context(tc.tile_pool(name="dwp", bufs=3))

        maxp_cache = {}
        avgp_cache = {}

        def pool_pass(x, out, tmp, op):
            tt = nc.vector.tensor_tensor
            tt(tmp[:, :, 1:31], x[:, :, 0:30], x[:, :, 1:31], op=op)
            tt(tmp[:, :, 1:31], tmp[:, :, 1:31], x[:, :, 2:32], op=op)
            tt(tmp[:, :, 0:1], x[:, :, 0:1], x[:, :, 1:2], op=op)
            tt(tmp[:, :, 31:32], x[:, :, 30:31], x[:, :, 31:32], op=op)
            tt(out[:, 1:31, :], tmp[:, 0:30, :], tmp[:, 1:31, :], op=op)
            tt(out[:, 1:31, :], out[:, 1:31, :], tmp[:, 2:32, :], op=op)
            tt(out[:, 0:1, :], tmp[:, 0:1, :], tmp[:, 1:2, :], op=op)
            tt(out[:, 31:32, :], tmp[:, 30:31, :], tmp[:, 31:32, :], op=op)

        def get_maxp(s):
            if s not in maxp_cache:
                tmp = scratch.tile([128, HH, WW], F32, tag="ptmp", name="ptmp", bufs=2)
                out = poolp.tile([128, HH, WW], F32, tag=f"maxp{s}", name=f"maxp{s}")
                pool_pass(states[s], out, tmp, mybir.AluOpType.max)
                maxp_cache[s] = out
            return maxp_cache[s]

        def get_avgp(s):
            if s not in avgp_cache:
                tmp = scratch.tile([128, HH, WW], F32, tag="ptmp", name="ptmp", bufs=2)
                out = poolp.tile([128, HH, WW], F32, tag=f"avgp{s}", name=f"avgp{s}")
                pool_pass(states[s], out, tmp, mybir.AluOpType.add)
                nc.vector.tensor_tensor(
                    out[:].rearrange("p a b -> p (a b)"),
                    out[:].rearrange("p a b -> p (a b)"),
                    rcnt[:],
                    op=mybir.AluOpType.mult,
                )
                avgp_cache[s] = out
            return avgp_cache[s]

        mpad_rot = [0]

        for i in range(STEPS):
            tgt = 2 + i
            step_edges = [ed for ed in plan["edges"] if ed["step"] == i]
            if os.environ.get("KERNEL_EDGE_ORDER", "src") == "dve":
                newest = 2 + i - 1

                def _dvework(ed):
                    tot = 0
                    for nm2 in ("sep3", "sep5", "dil3", "dil5"):
                        op2 = ed["ops"][nm2]
                        if op2 is None:
                            continue
                        for ent2 in op2["emit"]:
                            tot += len(ent2["dve"])
                    return tot

                step_edges = sorted(
                    step_edges,
                    key=lambda ed: (ed["src"] == newest, -_dvework(ed)),
                )
            n_acc = 0
            any_extra = False
            for ed in step_edges:
                for nm in ("sep3", "sep5", "dil3", "dil5"):
                    op = ed["ops"][nm]
                    if op is None:
                        continue
                    ent = op["emit"][-1]
                    n_acc += len(ent["pe_taps"]) + (1 if ent["dve"] else 0)
                if (
                    ed["ops"]["max"] != 0.0
                    or ed["ops"]["avg"] != 0.0
                    or ed["ops"]["skip"] != 0.0
                ):
                    any_extra = True

            acc = psum.tile([128, PIX], F32, tag="acc", name="acc") if n_acc else None
            extra = None
            if any_extra:
                extra = extrap.tile([128, PIX], F32, tag="extra", name="extra")
                nc.gpsimd.memset(extra[:], 0.0)
            acc_idx = [0, 0]

            def acc_mm(h, lhsT, rhs):
                nc.tensor.matmul(
                    acc[:, 512 * h : 512 * (h + 1)],
                    lhsT,
                    rhs,
                    start=(acc_idx[h] == 0),
                    stop=(acc_idx[h] == n_acc - 1),
                )
                acc_idx[h] += 1

            def dma_weights(ent):
                tiles = {}
                if ent["pe_taps"]:
                    ntap = len(ent["pe_taps"])
                    wt = wp.tile(
                        [128, ntap * 128],
                        F32R,
                        tag=f"w{ent['wclass']}",
                        name="wt",
                        bufs=int(os.environ.get("KERNEL_WBUFS", "4")),
                    )
                    nc.sync.dma_start(
                        wt[:],
                        d_wall[:, ent["pe_off"] * 128 : (ent["pe_off"] + ntap) * 128],
                    )
                    tiles["pe"] = wt
                if ent["dve"]:
                    wt = wp.tile([128, 128], F32R, tag="wpw", name="wtp", bufs=int(os.environ.get("KERNEL_PWBUFS", "8")))
                    nc.sync.dma_start(
                        wt[:],
                        d_wall[:, ent["pw_off"] * 128 : (ent["pw_off"] + 1) * 128],
                    )
                    tiles["pw"] = wt
                return tiles

            def emit_chain(ent, src_t, pad):
                if not ent["dve"]:
                    return
                dwacc = dwp.tile(
                    [128, HH, WW],
                    F32R,
                    tag="dwacc",
                    name="dwacc",
                    bufs=int(os.environ.get("KERNEL_DWBUFS", "4")),
                )
                for t_i, (dy, dx, col) in enumerate(ent["dve"]):
                    w3 = src_t[:, pad + dy : pad + dy + HH, pad + dx : pad + dx + WW]
                    if t_i == 0:
                        nc.scalar.activation(
                            dwacc[:], w3, AF.Copy, scale=dwtab[:, col : col + 1]
                        )
                    else:
                        nc.vector.scalar_tensor_tensor(
                            dwacc[:],
                            w3,
                            dwtab[:, col : col + 1],
                            dwacc[:],
                            op0=AOm,
                            op1=AOa,
                        )
                ent["_dwacc"] = dwacc

            def emit_layer(ent, tiles, src_t, pad, into_acc, mid_ps=None):
                n_mm = 2 * (len(ent["pe_taps"]) + (1 if ent["dve"] else 0))
                mm_i = [0]

                def do_mm(lhsT, rhs, h):
                    if into_acc:
                        acc_mm(h, lhsT, rhs)
                    else:
                        nc.tensor.matmul(
                            mid_ps[h][:],
                            lhsT,
                            rhs,
                            start=(mm_i[0] < 2),
                            stop=(mm_i[0] >= n_mm - 2),
                        )
                    mm_i[0] += 1

                dwacc = ent.pop("_dwacc", None)
                for t, (dy, dx) in enumerate(ent["pe_taps"]):
                    lhsT = tiles["pe"][:, t * 128 : (t + 1) * 128]
                    for h in range(2):
                        rhs = src_t[
                            :,
                            pad + dy + 16 * h : pad + dy + 16 * h + 16,
                            pad + dx : pad + dx + 32,
                        ]
                        do_mm(lhsT, rhs, h)
                if ent["dve"]:
                    df = dwacc[:].rearrange("p a b -> p (a b)")
                    for h in range(2):
                        do_mm(tiles["pw"], df[:, 512 * h : 512 * (h + 1)], h)

            for ed in step_edges:
                s = ed["src"]
                ops = ed["ops"]
                x = states[s]
                xf = x[:].rearrange("p a b -> p (a b)")
                stt_op = nc.vector.scalar_tensor_tensor

                live = [
                    nm for nm in ("sep3", "sep5", "dil3", "dil5") if ops[nm] is not None
                ]
                if live:
                    rp = rpads[ed["e"] % n_rpad]
                    nc.scalar.activation(rp[:, 4:36, 4:36], x[:], AF.Relu)
                    tiles = {
                        nm: [dma_weights(ent) for ent in ops[nm]["emit"]]
                        for nm in live
                    }
                    # DVE dw chains for rpad-fed layers first (high priority)
                    for nm in live:
                        emit_chain(ops[nm]["emit"][0], rp, 4)
                    # PE taps: dil layers straight into acc
                    for nm in ("dil3", "dil5"):
                        if nm in live:
                            emit_layer(ops[nm]["emit"][0], tiles[nm][0], rp, 4, True)
                    # sep layers: L1 -> mpad -> (L2 chain) -> L2 taps
                    for nm in ("sep3", "sep5"):
                        if nm not in live:
                            continue
                        op = ops[nm]
                        ent1, ent2 = op["emit"]
                        mid = [
                            psum.tile([128, 512], F32, tag="mid", name="mid", bufs=4)
                            for _ in range(2)
                        ]
                        emit_layer(ent1, tiles[nm][0], rp, 4, False, mid)
                        mpad = mpads[mpad_rot[0] % n_mpad]
                        mpad_rot[0] += 1
                        for h in range(2):
                            nc.scalar.activation(
                                mpad[:, 2 + 16 * h : 18 + 16 * h, 2:34],
                                mid[h][:].rearrange("p (a b) -> p a b", a=16),
                                AF.Relu,
                                bias=bias_ap(bias_cols[(ed["e"], nm)]),
                            )
                        emit_chain(ent2, mpad, 2)
                        emit_layer(ent2, tiles[nm][1], mpad, 2, True)

                # pool/skip contributions (not on the PE critical path)
                if ops["max"] != 0.0:
                    mp = get_maxp(s)
                    stt_op(
                        extra[:],
                        mp[:].rearrange("p a b -> p (a b)"),
                        ops["max"],
                        extra[:],
                        op0=AOm,
                        op1=AOa,
                    )
                if ops["avg"] != 0.0:
                    ap_ = get_avgp(s)
                    stt_op(
                        extra[:],
                        ap_[:].rearrange("p a b -> p (a b)"),
                        ops["avg"],
                        extra[:],
                        op0=AOm,
                        op1=AOa,
                    )
                if ops["skip"] != 0.0:
                    stt_op(extra[:], xf, ops["skip"], extra[:], op0=AOm, op1=AOa)

            assert acc_idx[0] == n_acc and acc_idx[1] == n_acc, (acc_idx, n_acc)

            stt = stp.tile([128, HH, WW], F32, tag=f"state{tgt}", name=f"state{tgt}")
            sb = bias_ap(2 + (tgt - 2))
            if acc is not None and extra is not None:
                sf = stt[:].rearrange("p a b -> p (a b)")
                for h in range(2):
                    nc.vector.scalar_tensor_tensor(
                        sf[:, 512 * h : 512 * (h + 1)],
                        acc[:, 512 * h : 512 * (h + 1)],
                        sb,
                        extra[:, 512 * h : 512 * (h + 1)],
                        op0=AOa,
                        op1=AOa,
                    )
            elif acc is not None:
                nc.scalar.activation(
                    stt[:],
                    acc[:].rearrange("p (a b) -> p a b", a=HH),
                    AF.Identity,
                    bias=sb,
                )
            elif extra is not None:
                nc.vector.tensor_scalar(
                    stt[:].rearrange("p a b -> p (a b)"), extra[:], sb, None, op0=AOa
                )
            else:
                # only hoisted biases contribute: state = broadcast(state_bias)
                nc.scalar.activation(
                    stt[:],
                    rcnt[:].rearrange("p (a b) -> p a b", a=HH),
                    AF.Identity,
                    bias=sb,
                    scale=0.0,
                )
            states.append(stt)

            so = stt[:].rearrange("p a b -> p (a b)")
            for h in range(2):
                nc.sync.dma_start(
                    d_out[i][:, 512 * h : 512 * (h + 1)], so[:, 512 * h : 512 * (h + 1)]
                )

    nc.compile()
    return nc


def _make_btab(plan):
    btab = np.zeros((128, 64), np.float32)
    btab[:, 0] = plan["bias0"]
    btab[:, 1] = plan["bias1"]
    for i in range(4):
        btab[:, 2 + i] = plan["state_bias"][2 + i]
    col = 6
    for ed in plan["edges"]:
        for nm in ("sep3", "sep5"):
            if ed["ops"][nm] is not None:
                btab[:, col] = ed["ops"][nm]["bias1"]
                col += 1
    return btab


def make_in_maps(plan, inputs):
    wpre = np.zeros((128, 1024), np.float32)
    wpre[:, 0:512] = (
        plan["wpre0"].reshape(4, 128, 128).transpose(1, 0, 2).reshape(128, 512)
    )
    wpre[:, 512:1024] = (
        plan["wpre1"].reshape(4, 128, 128).transpose(1, 0, 2).reshape(128, 512)
    )
    btab = _make_btab(plan)
    s0 = _f32(inputs["s0"]).reshape(B, 4, 128, PIX)
    s1 = _f32(inputs["s1"]).reshape(B, 4, 128, PIX)
    base = {
        "wall": np.ascontiguousarray(plan["wall"]),
        "wpre": wpre,
        "btab": btab,
        "dwtab": np.ascontiguousarray(plan["dwtab"]),
        "rcnt": plan["rcnt"],
    }
    return [
        {
            **base,
            "s0b": np.ascontiguousarray(s0[b]),
            "s1b": np.ascontiguousarray(s1[b]),
        }
        for b in range(B)
    ]


def kernel(**inputs):
    plan = build_plan(inputs)

    if os.environ.get("KERNEL_NUMPY") == "1":
        s0 = _f32(inputs["s0"]).reshape(B, C_PREV, PIX)
        s1 = _f32(inputs["s1"]).reshape(B, C_PREV, PIX)
        outs = []
        for b in range(B):
            r = run_plan_numpy(plan, s0[b], s1[b])
            outs.append(r.reshape(4 * C, HH, WW))
        return np.stack(outs).astype(np.float32)

    from concourse.bass_utils import run_bass_kernel_spmd

    nc = build_device_program(plan)
    in_maps = make_in_maps(plan, inputs)
    res = run_bass_kernel_spmd(nc, in_maps, core_ids=list(range(N_CORES)))
    out = np.stack([res.results[b]["out"].reshape(4 * C, HH, WW) for b in range(B)])
    return out.astype(np.float32)


# revision 26
# speedup vs baseline: 375.8688x; 1.0136x over previous
"""Trainium2 Bass kernel for nn_Cell_67894843015282 (DARTS-style NAS cell).

Strategy:
  - All routing/gating logic (masks, node_sum chain, sigmoid gates) depends only
    on small parameter tensors -> computed on host in f32 (jax-on-CPU to match
    the reference bit-for-bit on the step() comparisons).
  - BN affine + channel gates + op coefficients folded into the pointwise conv
    matrices (per-output-channel scale) and hoisted bias vectors; ops with a
    zero coefficient are skipped entirely.
  - Depthwise (x) pointwise conv pairs fused into per-tap 128x128 matmuls on the
    tensor engine (fp32r), accumulating across taps/ops/edges directly in PSUM.
    A balance solver spills some taps to the vector engine (per-partition-scalar
    multiply-accumulate chains) with the pointwise applied by one extra matmul.
  - Pools (max/avg 3x3) computed once per source state on the vector engine
    (separable passes); skip/pool contributions accumulated with
    scalar_tensor_tensor into an SBUF accumulator.
  - Data parallel over batch: 1 image per NeuronCore, 8 cores.
"""

import os

import numpy as np

B, C, HH, WW = 8, 128, 32, 32
PIX = HH * WW
C_PREV = 512
STEPS, N_EDGES, N_OPS = 4, 14, 8
N_CORES = 8

# cost-model constants (ns) for the balance solver
PE_TAP = 470.0  # two N=512 fp32r matmuls
PE_MM = 235.0
DVE_STT = 1222.0  # scalar_tensor_tensor on (128,1024)
DVE_TS = 689.0  # tensor_scalar f32 (single-src, 2x) -- unused, tap0 is on ACT
DVE_TT_BIG = 1222.0
DVE_TT_SMALL = 250.0
ACT_OP = 1257.0

# ---------------------------------------------------------------------------
# Host-side gating / fusion (the "plan")
# ---------------------------------------------------------------------------


def _f32(x):
    return np.asarray(x, dtype=np.float32)


def _gate_math(inputs):
    """Replicate the data-independent gating chain of the reference in f32.

    Uses jax on CPU when available so the step() threshold comparisons match
    the reference numerics exactly; falls back to numpy.
    """
    try:
        import jax

        cpu = jax.devices("cpu")[0]

        with jax.default_device(cpu):
            import jax.numpy as jnp

            return _gate_math_impl(jnp, jax.nn.sigmoid, inputs, to_np=np.asarray)
    except Exception:

        def np_sig(x):
            return 1.0 / (1.0 + np.exp(-np.asarray(x, np.float32), dtype=np.float32))

        return _gate_math_impl(np, np_sig, inputs, to_np=np.asarray)


def _gate_math_impl(xp, sig, inputs, to_np):
    f32 = np.float32
    weights2 = xp.asarray(inputs["weights2"], dtype=f32)
    thre = xp.asarray(inputs["thre"], dtype=f32)
    mask_default = xp.asarray(inputs["mask_default"])
    kernel_param = xp.asarray(inputs["kernel_param"], dtype=f32)
    mask_k_default = xp.asarray(inputs["mask_k_default"])
    mask_w_default = xp.asarray(inputs["mask_w_default"])
    kernel_pre = xp.asarray(inputs["kernel_pre"], dtype=f32)
    thre_pre = xp.asarray(inputs["thre_pre"], dtype=f32)

    def step(x):
        return (x > 0).astype(f32)

    mdf = mask_default.astype(f32)

    g0 = sig(kernel_pre[0])
    mk0 = step(g0 - thre_pre[0])
    gv0 = to_np(g0 * mk0).astype(f32)
    g1p = sig(kernel_pre[1])
    mk1 = step(g1p - thre_pre[1])
    gv1 = to_np(g1p * mk1).astype(f32)

    n_states = 2
    offset = 0
    m_all = np.zeros((N_EDGES, N_OPS), np.float32)
    for i in range(STEPS):
        n = n_states
        weight_sum = (weights2[offset : offset + n] * mdf[offset : offset + n]).sum()
        for j in range(n):
            e = offset + j
            ns = weight_sum
            m_list = []
            for k in range(N_OPS):
                w = weights2[e, k]
                md = mdf[e, k]
                m = xp.where(
                    md == 0, f32(0.0), xp.where(w != ns, step(w - thre[e, k, 0]), md)
                )
                cond = (md != 0) & (w != ns) & (m == 0)
                m_list.append(m)
                ns = xp.where(cond, ns - w, ns)
            m_vec = xp.stack(m_list)
            weight_sum = (
                weight_sum - (weights2[e] * mdf[e]).sum() + (weights2[e] * m_vec).sum()
            )
            m_all[e] = to_np(m_vec)
        offset += n
        n_states += 1

    coef = to_np(weights2).astype(f32) * m_all

    gates = to_np(sig(kernel_param)).astype(f32)
    t1 = to_np(thre[:, :, 1]).astype(f32)
    t2 = to_np(thre[:, :, 2]).astype(f32)
    mk = (gates - t1[:, :, None] > 0).astype(f32) * (to_np(mask_k_default) != 0)
    mw = (gates - t2[:, :, None] > 0).astype(f32) * (to_np(mask_w_default) != 0)
    return dict(
        gv0=gv0,
        gv1=gv1,
        coef=coef,
        gates=gates,
        mk=mk.astype(np.float32),
        mw=mw.astype(np.float32),
    )


TAPS3 = [(dy, dx) for dy in (-1, 0, 1) for dx in (-1, 0, 1)]
TAPS5 = [(dy, dx) for dy in (-2, -1, 0, 1, 2) for dx in (-2, -1, 0, 1, 2)]
TAPS3D = [(dy, dx) for dy in (-2, 0, 2) for dx in (-2, 0, 2)]
TAPS5D = [(dy, dx) for dy in (-4, -2, 0, 2, 4) for dx in (-4, -2, 0, 2, 4)]


def build_plan(inputs):
    g = _gate_math(inputs)
    coef = g["coef"]

    scale0 = _f32(inputs["pre0_g"]) * g["gv0"]
    bias0 = _f32(inputs["pre0_b"]) * g["gv0"]
    scale1 = _f32(inputs["pre1_g"]) * g["gv1"]
    bias1 = _f32(inputs["pre1_b"]) * g["gv1"]
    wpre0 = (_f32(inputs["pre0_w"]) * scale0[:, None]).T.copy()  # (512,128)
    wpre1 = (_f32(inputs["pre1_w"]) * scale1[:, None]).T.copy()

    state_of_edge = []
    for i in range(STEPS):
        for j in range(2 + i):
            state_of_edge.append((i, j))

    edges = []
    state_bias = np.zeros((6, C), np.float32)
    for e in range(N_EDGES):
        i, j = state_of_edge[e]
        tgt = 2 + i
        ops = {
            "max": float(coef[e, 1]),
            "avg": float(coef[e, 2]),
            "skip": float(coef[e, 3]),
        }
        for k, nm, taps in ((4, "sep3", TAPS3), (5, "sep5", TAPS5)):
            c = float(coef[e, k])
            if c == 0.0:
                ops[nm] = None
                continue
            gate = g["gates"][e, k]
            mk = g["mk"][e, k]
            mw = g["mw"][e, k]
            s1 = _f32(inputs[f"{nm}_g1"][e]) * gate * mk
            bb1 = _f32(inputs[f"{nm}_b1"][e]) * gate * mk
            s2 = c * _f32(inputs[f"{nm}_g2"][e]) * gate * mw
            bb2 = c * _f32(inputs[f"{nm}_b2"][e]) * gate * mw
            state_bias[tgt] += bb2
            if not s2.any() or not (s1.any() or bb1.any()):
                ops[nm] = None
                continue
            ops[nm] = dict(
                taps=taps,
                layers=[
                    dict(
                        dw=_f32(inputs[f"{nm}_dw1"][e]),
                        pw=_f32(inputs[f"{nm}_pw1"][e]),
                        scale=s1,
                    ),
                    dict(
                        dw=_f32(inputs[f"{nm}_dw2"][e]),
                        pw=_f32(inputs[f"{nm}_pw2"][e]),
                        scale=s2,
                    ),
                ],
                bias1=bb1,
            )
        for k, nm, taps in ((6, "dil3", TAPS3D), (7, "dil5", TAPS5D)):
            c = float(coef[e, k])
            if c == 0.0:
                ops[nm] = None
                continue
            gate = g["gates"][e, k]
            mk = g["mk"][e, k]
            s = c * _f32(inputs[f"{nm}_g"][e]) * gate * mk
            state_bias[tgt] += c * _f32(inputs[f"{nm}_b"][e]) * gate * mk
            if not s.any():
                ops[nm] = None
                continue
            ops[nm] = dict(
                taps=taps,
                layers=[
                    dict(
                        dw=_f32(inputs[f"{nm}_dw"][e]),
                        pw=_f32(inputs[f"{nm}_pw"][e]),
                        scale=s,
                    )
                ],
            )
        if ops["dil3"] is not None and ops["dil5"] is not None:
            # merge dil3 into dil5: every dil3 offset is also a dil5 offset and
            # both accumulate into the same psum -> sum the fused matrices.
            shared = [t for t in TAPS5D if t in TAPS3D]
            excl = [t for t in TAPS5D if t not in TAPS3D]
            lay5 = ops["dil5"]["layers"][0]
            # reorder dil5 dw columns to [shared, excl]
            order = [TAPS5D.index(t) for t in shared + excl]
            dw5 = lay5["dw"].reshape(C, 25)[:, order]
            ops["dil5"] = dict(
                taps=shared + excl,
                layers=[
                    dict(
                        dw=dw5,
                        pw=lay5["pw"],
                        scale=lay5["scale"],
                        merge=ops["dil3"]["layers"][0],
                        merge_taps=shared,
                    )
                ],
                max_donate=len(excl),
            )
            ops["dil3"] = None
        edges.append(dict(e=e, step=i, src=j, tgt=tgt, ops=ops))

    cnt1 = np.full(HH, 3.0, np.float32)
    cnt1[0] = cnt1[-1] = 2.0
    cnt = np.float32(1.0) / np.outer(cnt1, cnt1).astype(np.float32)
    rcnt = np.broadcast_to(cnt.reshape(1, PIX), (C, PIX)).copy()

    plan = dict(
        edges=edges,
        wpre0=wpre0,
        wpre1=wpre1,
        bias0=bias0,
        bias1=bias1,
        state_bias=state_bias,
        rcnt=rcnt,
    )
    _solve_balance(plan)
    _fuse_weights(plan)
    return plan


def _layer_units(plan):
    units = []
    for ed in plan["edges"]:
        for nm in ("sep3", "sep5", "dil3", "dil5"):
            op = ed["ops"][nm]
            if op is None:
                continue
            cap = op.get("max_donate", len(op["taps"]) - 1)
            for li in range(len(op["layers"])):
                units.append((ed, nm, li, op["taps"], cap))
    return units


def _solve_balance(plan):
    """Assign DVE-donated taps. Total donation count D and max chain depth are
    tunable; default D from the analytic balance."""
    units = _layer_units(plan)
    total_taps = sum(len(t) for _, _, _, t, _ in units)

    used_max, used_avg = set(), set()
    n_contrib = 0
    for ed in plan["edges"]:
        if ed["ops"]["max"] != 0.0:
            used_max.add(ed["src"])
            n_contrib += 1
        if ed["ops"]["avg"] != 0.0:
            used_avg.add(ed["src"])
            n_contrib += 1
        if ed["ops"]["skip"] != 0.0:
            n_contrib += 1
    dve_fixed = (
        len(used_max) * (4 * DVE_TT_BIG + 4 * DVE_TT_SMALL)
        + len(used_avg) * (5 * DVE_TT_BIG + 4 * DVE_TT_SMALL)
        + n_contrib * DVE_STT
        + 4 * 1350.0
    )
    pe_fixed = 16 * PE_MM

    D_env = os.environ.get("KERNEL_DVE_TAPS")
    if D_env is not None:
        D_target = int(D_env)
    else:
        # balance: PE = pe_fixed + (total - D)*PE_TAP + 2*PE_MM*nsplit
        # DVE = dve_fixed + (D - nsplit)*DVE_STT ; assume nsplit ~ D/12
        factor = float(os.environ.get("KERNEL_DVE_FACTOR", "1.18"))
        D_target = int(
            max(
                0.0,
                factor
                * (pe_fixed + total_taps * PE_TAP - dve_fixed)
                / (PE_TAP + DVE_STT),
            )
        )
    max_chain = int(os.environ.get("KERNEL_MAX_CHAIN", "12"))

    donors = sorted(range(len(units)), key=lambda i: -units[i][4])
    dve_k = [0] * len(units)
    D = 0
    di = 0
    live = [i for i in donors if units[i][4] > 0]
    fill_mode = os.environ.get("KERNEL_DONATE", "rr") == "fill"
    while live and D < D_target:
        i = live[di % len(live)] if not fill_mode else live[0]
        cap = min(len(units[i][3]) - 1, units[i][4], max_chain)
        if dve_k[i] >= cap:
            live.remove(i)
            continue
        dve_k[i] += 1
        D += 1
        if not fill_mode:
            di += 1
    for i, (ed, nm, li, taps, cap) in enumerate(units):
        ed["ops"][nm].setdefault("dve_k", {})[li] = dve_k[i]
    plan["n_dve_taps"] = D


def _fuse_weights(plan):
    wall_blocks = []
    wall_off = 0
    dw_cols = []

    def add_block(mat):
        nonlocal wall_off
        wall_blocks.append(mat)
        off = wall_off
        wall_off += mat.shape[1] // 128
        return off

    for ed in plan["edges"]:
        for nm in ("sep3", "sep5", "dil3", "dil5"):
            op = ed["ops"][nm]
            if op is None:
                continue
            taps = op["taps"]
            T = len(taps)
            op["emit"] = []
            for li, lay in enumerate(op["layers"]):
                k_dve = op["dve_k"][li]
                pe_taps = taps[: T - k_dve]
                dve_taps = taps[T - k_dve :]
                dwf = lay["dw"].reshape(C, T)
                pws = lay["pw"] * lay["scale"][:, None]  # (O, Cin)
                ent = dict(
                    pe_off=None,
                    pe_taps=pe_taps,
                    dve=[],
                    pw_off=None,
                    wclass=T,
                )
                if pe_taps:
                    idxs = list(range(T - k_dve))
                    blk3 = dwf[:, idxs][:, :, None] * pws.T[:, None, :]  # (C,T',O)
                    if "merge" in lay:
                        mlay = lay["merge"]
                        mdw = mlay["dw"].reshape(C, len(lay["merge_taps"]))
                        mpws = mlay["pw"] * mlay["scale"][:, None]
                        for mt, tap in enumerate(lay["merge_taps"]):
                            ti = pe_taps.index(tap)
                            blk3[:, ti, :] += mdw[:, mt : mt + 1] * mpws.T
                    blk = blk3.reshape(C, len(idxs) * 128)
                    ent["pe_off"] = add_block(blk.astype(np.float32))
                if dve_taps:
                    for t_i, (dy, dx) in enumerate(dve_taps):
                        col = len(dw_cols)
                        dw_cols.append(dwf[:, T - k_dve + t_i])
                        ent["dve"].append((dy, dx, col))
                    ent["pw_off"] = add_block(pws.T.copy().astype(np.float32))
                op["emit"].append(ent)

    plan["wall"] = (
        np.concatenate(wall_blocks, axis=1)
        if wall_blocks
        else np.zeros((C, 128), np.float32)
    )
    plan["dwtab"] = (
        np.stack(dw_cols, axis=1).astype(np.float32)
        if dw_cols
        else np.zeros((C, 1), np.float32)
    )


# ---------------------------------------------------------------------------
# Numpy executor (host model of the device plan; for correctness testing)
# ---------------------------------------------------------------------------


def run_plan_numpy(plan, s0, s1):
    wall = plan["wall"]
    dwtab = plan["dwtab"]

    def pad_img(x, pad, fill=0.0):
        out = np.full((C, HH + 2 * pad, WW + 2 * pad), fill, np.float32)
        out[:, pad : pad + HH, pad : pad + WW] = x
        return out

    def win(xpad, pad, dy, dx):
        return xpad[:, pad + dy : pad + dy + HH, pad + dx : pad + dx + WW].reshape(
            C, PIX
        )

    def layer_out(ent, xpad, pad):
        acc = np.zeros((C, PIX), np.float32)
        for t, (dy, dx) in enumerate(ent["pe_taps"]):
            lhsT = wall[:, (ent["pe_off"] + t) * 128 : (ent["pe_off"] + t + 1) * 128]
            acc += lhsT.T @ win(xpad, pad, dy, dx)
        if ent["dve"]:
            dwacc = np.zeros((C, PIX), np.float32)
            for dy, dx, col in ent["dve"]:
                dwacc += dwtab[:, col : col + 1] * win(xpad, pad, dy, dx)
            lhsT = wall[:, ent["pw_off"] * 128 : (ent["pw_off"] + 1) * 128]
            acc += lhsT.T @ dwacc
        return acc

    states = []
    for s, w, bia in (
        (s0, plan["wpre0"], plan["bias0"]),
        (s1, plan["wpre1"], plan["bias1"]),
    ):
        r = np.maximum(s, 0.0)
        h = w.T @ r + bia[:, None]
        states.append(h.astype(np.float32))

    for i in range(STEPS):
        tgt = 2 + i
        acc = np.zeros((C, PIX), np.float32)
        acc += plan["state_bias"][tgt][:, None]
        for ed in plan["edges"]:
            if ed["step"] != i:
                continue
            x = states[ed["src"]].reshape(C, HH, WW)
            ops = ed["ops"]
            if ops["max"] != 0.0:
                xm = pad_img(x, 1, -np.inf)
                m = np.full((C, HH, WW), -np.inf, np.float32)
                for dy in (-1, 0, 1):
                    for dx in (-1, 0, 1):
                        m = np.maximum(
                            m, xm[:, 1 + dy : 1 + dy + HH, 1 + dx : 1 + dx + WW]
                        )
                acc += ops["max"] * m.reshape(C, PIX)
            if ops["avg"] != 0.0:
                xa = pad_img(x, 1, 0.0)
                ssum = np.zeros((C, HH, WW), np.float32)
                for dy in (-1, 0, 1):
                    for dx in (-1, 0, 1):
                        ssum += xa[:, 1 + dy : 1 + dy + HH, 1 + dx : 1 + dx + WW]
                acc += ops["avg"] * (ssum.reshape(C, PIX) * plan["rcnt"])
            if ops["skip"] != 0.0:
                acc += ops["skip"] * x.reshape(C, PIX)
            rp = pad_img(np.maximum(x, 0.0), 4)
            for nm in ("sep3", "sep5"):
                op = ops[nm]
                if op is None:
                    continue
                mid = layer_out(op["emit"][0], rp, 4)
                mid = np.maximum(mid + op["bias1"][:, None], 0.0)
                mp = pad_img(mid.reshape(C, HH, WW), 2)
                acc += layer_out(op["emit"][1], mp, 2)
            for nm in ("dil3", "dil5"):
                op = ops[nm]
                if op is None:
                    continue
                acc += layer_out(op["emit"][0], rp, 4)
        states.append(acc)

    return np.stack(states[2:], axis=0)


# ---------------------------------------------------------------------------
# Bass device program
# ---------------------------------------------------------------------------


def build_device_program(plan):
    from contextlib import ExitStack

    import concourse.bacc as bacc
    import concourse.mybir as mybir
    import concourse.tile as tile

    F32 = mybir.dt.float32
    F32R = mybir.dt.float32r
    AO = mybir.AluOpType
    AF = mybir.ActivationFunctionType
    AOm, AOa = AO.mult, AO.add

    wall_np = plan["wall"]
    n_wall_taps = wall_np.shape[1] // 128
    n_dw = plan["dwtab"].shape[1]

    nc = bacc.Bacc("TRN2", target_bir_lowering=False, debug=False)
    d_s0 = nc.dram_tensor("s0b", [4, 128, PIX], F32, kind="ExternalInput").ap()
    d_s1 = nc.dram_tensor("s1b", [4, 128, PIX], F32, kind="ExternalInput").ap()
    d_wall = nc.dram_tensor(
        "wall", [128, n_wall_taps * 128], F32R, kind="ExternalInput"
    ).ap()
    d_wpre = nc.dram_tensor("wpre", [128, 1024], F32R, kind="ExternalInput").ap()
    d_btab = nc.dram_tensor("btab", [128, 64], F32, kind="ExternalInput").ap()
    d_dwtab = nc.dram_tensor("dwtab", [128, n_dw], F32, kind="ExternalInput").ap()
    d_rcnt = nc.dram_tensor("rcnt", [128, PIX], F32, kind="ExternalInput").ap()
    d_out = nc.dram_tensor("out", [4, 128, PIX], F32, kind="ExternalOutput").ap()

    bias_cols = {}
    next_bias = 6
    for ed in plan["edges"]:
        for nm in ("sep3", "sep5"):
            if ed["ops"][nm] is not None:
                bias_cols[(ed["e"], nm)] = next_bias
                next_bias += 1
    assert next_bias <= 64

    used_max, used_avg = set(), set()
    for ed in plan["edges"]:
        if ed["ops"]["max"] != 0.0:
            used_max.add(ed["src"])
        if ed["ops"]["avg"] != 0.0:
            used_avg.add(ed["src"])

    with tile.TileContext(nc) as tc, ExitStack() as ctx:
        const = ctx.enter_context(tc.tile_pool(name="const", bufs=1))
        stp = ctx.enter_context(tc.tile_pool(name="stp", bufs=1))
        poolp = ctx.enter_context(tc.tile_pool(name="poolp", bufs=1))
        padp = ctx.enter_context(tc.tile_pool(name="padp", bufs=1))
        extrap = ctx.enter_context(tc.tile_pool(name="extrap", bufs=3))
        psum = ctx.enter_context(tc.tile_pool(name="psum", bufs=2, space="PSUM"))

        n_rpad = int(os.environ.get("KERNEL_NRPAD", "3"))
        n_mpad = int(os.environ.get("KERNEL_NMPAD", "4"))
        rpads = []
        for ri in range(n_rpad):
            t = padp.tile([128, 40, 40], F32R, tag=f"rpad{ri}", name=f"rpad{ri}")
            nc.gpsimd.memset(t[:].bitcast(F32), 0.0)
            rpads.append(t)

        mpads = []
        for mi in range(n_mpad):
            t = padp.tile([128, 36, 36], F32R, tag=f"mpad{mi}", name=f"mpad{mi}")
            nc.gpsimd.memset(t[:].bitcast(F32), 0.0)
            mpads.append(t)

        btab = const.tile([128, 64], F32, tag="btab", name="btab")
        nc.gpsimd.dma_start(btab[:], d_btab)
        rcnt = const.tile([128, PIX], F32, tag="rcnt", name="rcnt")
        nc.gpsimd.dma_start(rcnt[:], d_rcnt)
        dwtab = const.tile([128, n_dw], F32, tag="dwtab", name="dwtab")
        nc.gpsimd.dma_start(dwtab[:], d_dwtab)

        def bias_ap(col):
            return btab[:, col : col + 1]

        # ---- preprocess in its own (released-early) pool
        states = []
        with tc.tile_pool(name="prep", bufs=1) as prep:
            wpre = prep.tile([128, 1024], F32R, tag="wpre", name="wpre")
            nc.sync.dma_start(wpre[:], d_wpre)
            for si, dsrc in enumerate((d_s0, d_s1)):
                rel = {}
                for h in range(2):
                    for kk in range(4):
                        st = prep.tile(
                            [128, 512], F32, tag="stage", name="stage", bufs=4
                        )
                        nc.sync.dma_start(st[:], dsrc[kk][:, 512 * h : 512 * (h + 1)])
                        rl = prep.tile(
                            [128, 512], F32R, tag="relu", name="relu", bufs=10
                        )
                        if kk % 2 == 0:
                            nc.vector.tensor_scalar(rl[:], st[:], 0.0, None, op0=AO.max)
                        else:
                            nc.scalar.activation(rl[:], st[:], AF.Relu)
                        rel[(kk, h)] = rl
                ps = psum.tile([128, PIX], F32, tag="acc", name="pre_acc")
                for h in range(2):
                    for kk in range(4):
                        nc.tensor.matmul(
                            ps[:, 512 * h : 512 * (h + 1)],
                            wpre[:, 512 * si + 128 * kk : 512 * si + 128 * (kk + 1)],
                            rel[(kk, h)][:],
                            start=(kk == 0),
                            stop=(kk == 3),
                        )
                stt = stp.tile(
                    [128, HH, WW], F32, tag=f"state{si}", name=f"state{si}"
                )
                nc.scalar.activation(
                    stt[:],
                    ps[:].rearrange("p (a b) -> p a b", a=HH),
                    AF.Identity,
                    bias=bias_ap(si),
                )
                states.append(stt)

        scratch = ctx.enter_context(tc.tile_pool(name="scratch", bufs=3))
        wp = ctx.enter_context(tc.tile_pool(name="wp", bufs=3))
        dwp = ctx.enter_context(tc.tile_pool(name="dwp", bufs=3))

        maxp_cache = {}
        avgp_cache = {}

        def pool_pass(x, out, tmp, op):
            tt = nc.vector.tensor_tensor
            tt(tmp[:, :, 1:31], x[:, :, 0:30], x[:, :, 1:31], op=op)
            tt(tmp[:, :, 1:31], tmp[:, :, 1:31], x[:, :, 2:32], op=op)
            tt(tmp[:, :, 0:1], x[:, :, 0:1], x[:, :, 1:2], op=op)
            tt(tmp[:, :, 31:32], x[:, :, 30:31], x[:, :, 31:32], op=op)
            tt(out[:, 1:31, :], tmp[:, 0:30, :], tmp[:, 1:31, :], op=op)
            tt(out[:, 1:31, :], out[:, 1:31, :], tmp[:, 2:32, :], op=op)
            tt(out[:, 0:1, :], tmp[:, 0:1, :], tmp[:, 1:2, :], op=op)
            tt(out[:, 31:32, :], tmp[:, 30:31, :], tmp[:, 31:32, :], op=op)

        def get_maxp(s):
            if s not in maxp_cache:
                tmp = scratch.tile([128, HH, WW], F32, tag="ptmp", name="ptmp", bufs=2)
                out = poolp.tile([128, HH, WW], F32, tag=f"maxp{s}", name=f"maxp{s}")
                pool_pass(states[s], out, tmp, mybir.AluOpType.max)
                maxp_cache[s] = out
            return maxp_cache[s]

        def get_avgp(s):
            if s not in avgp_cache:
                tmp = scratch.tile([128, HH, WW], F32, tag="ptmp", name="ptmp", bufs=2)
                out = poolp.tile([128, HH, WW], F32, tag=f"avgp{s}", name=f"avgp{s}")
                pool_pass(states[s], out, tmp, mybir.AluOpType.add)
                nc.vector.tensor_tensor(
                    out[:].rearrange("p a b -> p (a b)"),
                    out[:].rearrange("p a b -> p (a b)"),
                    rcnt[:],
                    op=mybir.AluOpType.mult,
                )
                avgp_cache[s] = out
            return avgp_cache[s]

        mpad_rot = [0]

        for i in range(STEPS):
            tgt = 2 + i
            step_edges = [ed for ed in plan["edges"] if ed["step"] == i]
            if os.environ.get("KERNEL_EDGE_ORDER", "src") == "dve":
                newest = 2 + i - 1

                def _dvework(ed):
                    tot = 0
                    for nm2 in ("sep3", "sep5", "dil3", "dil5"):
                        op2 = ed["ops"][nm2]
                        if op2 is None:
                            continue
                        for ent2 in op2["emit"]:
                            tot += len(ent2["dve"])
                    return tot

                step_edges = sorted(
                    step_edges,
                    key=lambda ed: (ed["src"] == newest, -_dvework(ed)),
                )
            n_acc = 0
            any_extra = False
            for ed in step_edges:
                for nm in ("sep3", "sep5", "dil3", "dil5"):
                    op = ed["ops"][nm]
                    if op is None:
                        continue
                    ent = op["emit"][-1]
                    n_acc += len(ent["pe_taps"]) + (1 if ent["dve"] else 0)
                if (
                    ed["ops"]["max"] != 0.0
                    or ed["ops"]["avg"] != 0.0
                    or ed["ops"]["skip"] != 0.0
                ):
                    any_extra = True

            acc = psum.tile([128, PIX], F32, tag="acc", name="acc") if n_acc else None
            extra = None
            if any_extra:
                extra = extrap.tile([128, PIX], F32, tag="extra", name="extra")
                nc.gpsimd.memset(extra[:], 0.0)
            acc_idx = [0, 0]

            def acc_mm(h, lhsT, rhs):
                nc.tensor.matmul(
                    acc[:, 512 * h : 512 * (h + 1)],
                    lhsT,
                    rhs,
                    start=(acc_idx[h] == 0),
                    stop=(acc_idx[h] == n_acc - 1),
                )
                acc_idx[h] += 1

            def dma_weights(ent):
                tiles = {}
                if ent["pe_taps"]:
                    ntap = len(ent["pe_taps"])
                    wt = wp.tile(
                        [128, ntap * 128],
                        F32R,
                        tag=f"w{ent['wclass']}",
                        name="wt",
                        bufs=int(os.environ.get("KERNEL_WBUFS", "4")),
                    )
                    nc.sync.dma_start(
                        wt[:],
                        d_wall[:, ent["pe_off"] * 128 : (ent["pe_off"] + ntap) * 128],
                    )
                    tiles["pe"] = wt
                if ent["dve"]:
                    wt = wp.tile([128, 128], F32R, tag="wpw", name="wtp", bufs=int(os.environ.get("KERNEL_PWBUFS", "8")))
                    nc.sync.dma_start(
                        wt[:],
                        d_wall[:, ent["pw_off"] * 128 : (ent["pw_off"] + 1) * 128],
                    )
                    tiles["pw"] = wt
                return tiles

            def emit_chain(ent, src_t, pad):
                if not ent["dve"]:
                    return
                dwacc = dwp.tile(
                    [128, HH, WW],
                    F32R,
                    tag="dwacc",
                    name="dwacc",
                    bufs=int(os.environ.get("KERNEL_DWBUFS", "4")),
                )
                for t_i, (dy, dx, col) in enumerate(ent["dve"]):
                    w3 = src_t[:, pad + dy : pad + dy + HH, pad + dx : pad + dx + WW]
                    if t_i == 0:
                        nc.scalar.activation(
                            dwacc[:], w3, AF.Copy, scale=dwtab[:, col : col + 1]
                        )
                    else:
                        nc.vector.scalar_tensor_tensor(
                            dwacc[:],
                            w3,
                            dwtab[:, col : col + 1],
                            dwacc[:],
                            op0=AOm,
                            op1=AOa,
                        )
                ent["_dwacc"] = dwacc

            def emit_layer(ent, tiles, src_t, pad, into_acc, mid_ps=None):
                n_mm = 2 * (len(ent["pe_taps"]) + (1 if ent["dve"] else 0))
                mm_i = [0]

                def do_mm(lhsT, rhs, h):
                    if into_acc:
                        acc_mm(h, lhsT, rhs)
                    else:
                        nc.tensor.matmul(
                            mid_ps[h][:],
                            lhsT,
                            rhs,
                            start=(mm_i[0] < 2),
                            stop=(mm_i[0] >= n_mm - 2),
                        )
                    mm_i[0] += 1

                dwacc = ent.pop("_dwacc", None)
                for t, (dy, dx) in enumerate(ent["pe_taps"]):
                    lhsT = tiles["pe"][:, t * 128 : (t + 1) * 128]
                    for h in range(2):
                        rhs = src_t[
                            :,
                            pad + dy + 16 * h : pad + dy + 16 * h + 16,
                            pad + dx : pad + dx + 32,
                        ]
                        do_mm(lhsT, rhs, h)
                if ent["dve"]:
                    df = dwacc[:].rearrange("p a b -> p (a b)")
                    for h in range(2):
                        do_mm(tiles["pw"], df[:, 512 * h : 512 * (h + 1)], h)

            for ed in step_edges:
                s = ed["src"]
                ops = ed["ops"]
                x = states[s]
                xf = x[:].rearrange("p a b -> p (a b)")
                stt_op = nc.vector.scalar_tensor_tensor

                live = [
                    nm for nm in ("sep3", "sep5", "dil3", "dil5") if ops[nm] is not None
                ]
                if live:
                    rp = rpads[ed["e"] % n_rpad]
                    nc.scalar.activation(rp[:, 4:36, 4:36], x[:], AF.Relu)
                    tiles = {
                        nm: [dma_weights(ent) for ent in ops[nm]["emit"]]
                        for nm in live
                    }
                    # DVE dw chains for rpad-fed layers first (high priority)
                    for nm in live:
                        emit_chain(ops[nm]["emit"][0], rp, 4)
                    # PE taps: dil layers straight into acc
                    for nm in ("dil3", "dil5"):
                        if nm in live:
                            emit_layer(ops[nm]["emit"][0], tiles[nm][0], rp, 4, True)
                    # sep layers: L1 -> mpad -> (L2 chain) -> L2 taps
                    for nm in ("sep3", "sep5"):
                        if nm not in live:
                            continue
                        op = ops[nm]
                        ent1, ent2 = op["emit"]
                        mid = [
                            psum.tile([128, 512], F32, tag="mid", name="mid", bufs=4)
                            for _ in range(2)
                        ]
                        emit_layer(ent1, tiles[nm][0], rp, 4, False, mid)
                        mpad = mpads[mpad_rot[0] % n_mpad]
                        mpad_rot[0] += 1
                        for h in range(2):
                            nc.scalar.activation(
                                mpad[:, 2 + 16 * h : 18 + 16 * h, 2:34],
                                mid[h][:].rearrange("p (a b) -> p a b", a=16),
                                AF.Relu,
                                bias=bias_ap(bias_cols[(ed["e"], nm)]),
                            )
                        emit_chain(ent2, mpad, 2)
                        emit_layer(ent2, tiles[nm][1], mpad, 2, True)

                # pool/skip contributions (not on the PE critical path)
                if ops["max"] != 0.0:
                    mp = get_maxp(s)
                    stt_op(
                        extra[:],
                        mp[:].rearrange("p a b -> p (a b)"),
                        ops["max"],
                        extra[:],
                        op0=AOm,
                        op1=AOa,
                    )
                if ops["avg"] != 0.0:
                    ap_ = get_avgp(s)
                    stt_op(
                        extra[:],
                        ap_[:].rearrange("p a b -> p (a b)"),
                        ops["avg"],
                        extra[:],
                        op0=AOm,
                        op1=AOa,
                    )
                if ops["skip"] != 0.0:
                    stt_op(extra[:], xf, ops["skip"], extra[:], op0=AOm, op1=AOa)

            assert acc_idx[0] == n_acc and acc_idx[1] == n_acc, (acc_idx, n_acc)

            stt = stp.tile([128, HH, WW], F32, tag=f"state{tgt}", name=f"state{tgt}")
            sb = bias_ap(2 + (tgt - 2))
            if acc is not None and extra is not None:
                sf = stt[:].rearrange("p a b -> p (a b)")
                for h in range(2):
                    nc.vector.scalar_tensor_tensor(
                        sf[:, 512 * h : 512 * (h + 1)],
                        acc[:, 512 * h : 512 * (h + 1)],
                        sb,
                        extra[:, 512 * h : 512 * (h + 1)],
                        op0=AOa,
                        op1=AOa,
                    )
            elif acc is not None:
                nc.scalar.activation(
                    stt[:],
                    acc[:].rearrange("p (a b) -> p a b", a=HH),
                    AF.Identity,
                    bias=sb,
                )
            elif extra is not None:
                nc.vector.tensor_scalar(
                    stt[:].rearrange("p a b -> p (a b)"), extra[:], sb, None, op0=AOa
                )
            else:
                # only hoisted biases contribute: state = broadcast(state_bias)
                nc.scalar.activation(
                    stt[:],
                    rcnt[:].rearrange("p (a b) -> p a b", a=HH),
                    AF.Identity,
                    bias=sb,
                    scale=0.0,
                )
            states.append(stt)

            so = stt[:].rearrange("p a b -> p (a b)")
            for h in range(2):
                nc.sync.dma_start(
                    d_out[i][:, 512 * h : 512 * (h + 1)], so[:, 512 * h : 512 * (h + 1)]
                )

    nc.compile()
    return nc


def _make_btab(plan):
    btab = np.zeros((128, 64), np.float32)
    btab[:, 0] = plan["bias0"]
    btab[:, 1] = plan["bias1"]
    for i in range(4):
        btab[:, 2 + i] = plan["state_bias"][2 + i]
    col = 6
    for ed in plan["edges"]:
        for nm in ("sep3", "sep5"):
            if ed["ops"][nm] is not None:
                btab[:, col] = ed["ops"][nm]["bias1"]
                col += 1
    return btab


def make_in_maps(plan, inputs):
    wpre = np.zeros((128, 1024), np.float32)
    wpre[:, 0:512] = (
        plan["wpre0"].reshape(4, 128, 128).transpose(1, 0, 2).reshape(128, 512)
    )
    wpre[:, 512:1024] = (
        plan["wpre1"].reshape(4, 128, 128).transpose(1, 0, 2).reshape(128, 512)
    )
    btab = _make_btab(plan)
    s0 = _f32(inputs["s0"]).reshape(B, 4, 128, PIX)
    s1 = _f32(inputs["s1"]).reshape(B, 4, 128, PIX)
    base = {
        "wall": np.ascontiguousarray(plan["wall"]),
        "wpre": wpre,
        "btab": btab,
        "dwtab": np.ascontiguousarray(plan["dwtab"]),
        "rcnt": plan["rcnt"],
    }
    return [
        {
            **base,
            "s0b": np.ascontiguousarray(s0[b]),
            "s1b": np.ascontiguousarray(s1[b]),
        }
        for b in range(B)
    ]


def kernel(**inputs):
    plan = build_plan(inputs)

    if os.environ.get("KERNEL_NUMPY") == "1":
        s0 = _f32(inputs["s0"]).reshape(B, C_PREV, PIX)
        s1 = _f32(inputs["s1"]).reshape(B, C_PREV, PIX)
        outs = []
        for b in range(B):
            r = run_plan_numpy(plan, s0[b], s1[b])
            outs.append(r.reshape(4 * C, HH, WW))
        return np.stack(outs).astype(np.float32)

    from concourse.bass_utils import run_bass_kernel_spmd

    nc = build_device_program(plan)
    in_maps = make_in_maps(plan, inputs)
    res = run_bass_kernel_spmd(nc, in_maps, core_ids=list(range(N_CORES)))
    out = np.stack([res.results[b]["out"].reshape(4 * C, HH, WW) for b in range(B)])
    return out.astype(np.float32)
